# revision 26
# baseline (speedup 1.0000x reference)
"""Trainium2 Bass kernel for nn_DAWNBlock (8-core SPMD), v5.

Decomposition (validated in numpy: proto_check.py, quant_check.py):
  - Token-sharded with batch-interleaved ownership: core c owns global tokens
    [256c, 256c+256) of batch 0 AND [256c, 256c+256) of batch 1 (T=512).
    LN1, features, restores (Q/K/V), LN2 + knowledge run token-local.
  - Head-sharded attention: core c owns d-cols [128c, 128c+128) (heads
    {2c, 2c+1}). One A2A reshards Q^T+K^T (1MB fp8), one reshards V (0.5MB).
    Attention output returns via TWO 0.25MB A2As (one per batch) so batch 0's
    resharding and its W_O hide under batch 1's attention.
  - fp8e4m3 + DoubleRow (K=256 pairs, 2x PE rate) for features, restores,
    PV, W_O, knowledge. Scores fp8 K=64 packed as concurrent row-group pairs.
    All-fp8 quantization study: 4e-3 rel err vs the 2e-2 gate.
  - LN gains folded into the f-pools host-side (bias asserted zero); LN is
    stats + one ACT apply. Routing weights: feature PSUM banks combined via
    DVE scalar_tensor_tensor; restores use A[n] = hT * wbc[n] (PE-broadcast
    rows), h transposed once.
  - Causal softmax without max-subtraction; denominator via ones-column in V;
    o_ps evacuated to SBUF immediately to release PSUM banks.
"""
import sys

sys.path.insert(0, "/opt/trn_rl_repo")

import os
import numpy as np
import ml_dtypes
import concourse.bass as bass
import concourse.mybir as mybir
import concourse.tile as tile
from concourse import bacc
from concourse.bass_utils import run_bass_kernel_spmd
from concourse.masks import make_identity

B, S, D, H, R, N, KR = 2, 2048, 1024, 16, 256, 8, 128
DH = D // H           # 64
BS = B * S            # 4096
NC = 8
T = BS // NC          # 512 tokens per core (256 per batch)
TB = T // B           # 256 tokens per (core, batch)
P = 128
TT = T // P           # 4 token tiles per core
DC = D // P           # 8 d chunks
DCP = DC // 2         # 4 d chunk-pairs (DoubleRow)
RC = R // P           # 2 r chunks
EPS = 1e-5

STAGES = int(os.environ.get("BASS_STAGES", "5"))
F32 = mybir.dt.float32
BF = mybir.dt.bfloat16
F8 = mybir.dt.float8e4
DR = mybir.MatmulPerfMode.DoubleRow
AF = mybir.ActivationFunctionType
OP = mybir.AluOpType


def _layernorm(nc, cpool, x_sb, nx_sb, eps_tile, tag):
    """nx = (x - mean(x)) * rsqrt(var + eps) for one [128, D] tile.
    (LN gain folded into downstream pools host-side; bias asserted zero.)"""
    s = cpool.tile([P, 1], F32, tag="ln_s", name=f"{tag}_s")
    nm = cpool.tile([P, 1], F32, tag="ln_nm", name=f"{tag}_nm")
    sq = cpool.tile([P, D], F32, tag="ln_sq", name=f"{tag}_sq")
    ssq = cpool.tile([P, 1], F32, tag="ln_ssq", name=f"{tag}_ssq")
    rs = cpool.tile([P, 1], F32, tag="ln_rs", name=f"{tag}_rs")
    nmrs = cpool.tile([P, 1], F32, tag="ln_nmrs", name=f"{tag}_nmrs")
    nc.vector.reduce_sum(s[:], x_sb[:], axis=mybir.AxisListType.X)
    nc.vector.tensor_scalar_mul(nm[:], s[:], -1.0 / D)
    nc.scalar.activation(sq[:], x_sb[:], AF.Square, bias=nm[:], accum_out=ssq[:])
    nc.scalar.activation(rs[:], ssq[:], AF.Abs_reciprocal_sqrt,
                         bias=eps_tile[:], scale=1.0 / D)
    nc.vector.tensor_mul(nmrs[:], nm[:], rs[:])
    nc.scalar.activation(nx_sb[:], x_sb[:], AF.Identity, bias=nmrs[:], scale=rs[:])


def _build():
    nc = bacc.Bacc("TRN2", target_bir_lowering=False, debug=False, num_devices=NC)

    def di(name, shape, dt=F8):
        return nc.dram_tensor(name, shape, dt, kind="ExternalInput").ap()

    x_in = di("x_sh", [T, D], F32)
    wcol_in = {k: di(k, [T, N], F32) for k in ["wfq", "wfk", "wfv", "wkf"]}
    wrow_in = {k: di(k, [N, T], BF) for k in ["wrqT", "wrkT", "wrvT", "wkrT"]}
    fqk_in = di("fqk_p", [N, D, R])
    fv_in = di("fv_p", [N, D, R])
    rqk_in = di("rqk_p", [N, R, D])
    rv_in = di("rv_p", [N, R, D])
    fkn_in = di("fkn_p", [N, D, KR])
    rkn_in = di("rkn_p", [N, KR, D])
    wo_in = di("wo_p", [D, D])       # = W_O.T
    masku_in = di("masku", [P, P])
    out_ap = nc.dram_tensor("out_sh", [T, D], F32, kind="ExternalOutput").ap()

    with tile.TileContext(nc) as tc:
        from contextlib import ExitStack
        with ExitStack() as ctx:
            const = ctx.enter_context(tc.tile_pool(name="const", bufs=1))
            cpool = ctx.enter_context(tc.tile_pool(name="scratch", bufs=2))
            dram = ctx.enter_context(tc.tile_pool(name="dram", bufs=1, space="DRAM"))

            # ---------- pools (stack order: long-lived first) ----------
            xpool = ctx.enter_context(tc.tile_pool(name="xpool", bufs=1))
            x_t = [xpool.tile([P, D], F32, tag=f"x{tt}", name=f"x{tt}") for tt in range(TT)]

            cprep = ctx.enter_context(tc.tile_pool(name="cprep", bufs=1))
            wo_sb = cprep.tile([P, DCP, 2, D], F8, tag="wo_sb", name="wo_sb")
            fkn_sb = cprep.tile([P, DCP, 2, N, KR], F8, tag="fkn_sb", name="fkn_sb")
            rkn_sb = cprep.tile([P, N, D], F8, tag="rkn_sb", name="rkn_sb")
            wbc_kr = []

            hpool = ctx.enter_context(tc.tile_pool(name="hpool", bufs=1))
            h_q = [hpool.tile([P, R], F32, tag=f"hq{tt}", name=f"hq{tt}") for tt in range(TT)]
            h_k = [hpool.tile([P, R], F32, tag=f"hk{tt}", name=f"hk{tt}") for tt in range(TT)]
            h_v = [hpool.tile([P, R], F32, tag=f"hv{tt}", name=f"hv{tt}") for tt in range(TT)]
            hbf_q = [hpool.tile([P, R], BF, tag=f"hbq{tt}", name=f"hbq{tt}") for tt in range(TT)]
            hbf_k = [hpool.tile([P, R], BF, tag=f"hbk{tt}", name=f"hbk{tt}") for tt in range(TT)]
            hbf_v = [hpool.tile([P, R], BF, tag=f"hbv{tt}", name=f"hbv{tt}") for tt in range(TT)]

            from contextlib import ExitStack as _ES
            rp_ctx = _ES()
            rp = rp_ctx.enter_context(tc.tile_pool(name="rp", bufs=1))
            rqk_sb = rp.tile([P, N, RC, D], F8, tag="rqk_sb", name="rqk_sb")
            rv_sb = rp.tile([P, N, RC, D], F8, tag="rv_sb", name="rv_sb")

            fw_ctx = _ES()
            fwpool = fw_ctx.enter_context(tc.tile_pool(name="fwpool", bufs=1))
            fqk_sb = [fwpool.tile([P, 2, N, R], F8, tag=f"fqk{dcp}", name=f"fqk{dcp}")
                      for dcp in range(DCP)]
            fv_sb = [fwpool.tile([P, 2, N, R], F8, tag=f"fv{dcp}", name=f"fv{dcp}")
                     for dcp in range(DCP)]

            # ---------- big prefetches in priority order ----------
            for tt in range(TT):
                nc.sync.dma_start(x_t[tt][:], x_in[P * tt:P * (tt + 1), :])
            for dcp in range(DCP):
                for i in range(2):
                    dc = 2 * dcp + i
                    nc.sync.dma_start(
                        fqk_sb[dcp][:, i],
                        fqk_in[:, P * dc:P * (dc + 1), :].rearrange("n p r -> p n r"))
                    nc.sync.dma_start(
                        fv_sb[dcp][:, i],
                        fv_in[:, P * dc:P * (dc + 1), :].rearrange("n p r -> p n r"))
            for n in range(N):
                nc.sync.dma_start(
                    rqk_sb[:, n], rqk_in[n].rearrange("(rc p) d -> p rc d", p=P))
                nc.sync.dma_start(
                    rv_sb[:, n], rv_in[n].rearrange("(rc p) d -> p rc d", p=P))
            # tiny warm-up collective: pays the first-collective ncfw latency
            dumm_b = dram.tile([NC * 8, 8], F32, tag="dumm_b", name="dumm_b")
            dumm_o = dram.tile([NC * 8, 8], F32, tag="dumm_o", name="dumm_o")
            nc.gpsimd.collective_compute(
                "AllToAll", OP.bypass, replica_groups=[list(range(NC))],
                ins=[dumm_b.opt()], outs=[dumm_o.opt()])

            ident_f = const.tile([P, P], F32, tag="ident_f", name="ident_f")
            make_identity(nc, ident_f)
            identb = const.tile([P, P], BF, tag="identb", name="identb")
            nc.vector.tensor_copy(identb[:], ident_f[:])
            ones_bf = const.tile([1, P], BF, tag="ones_bf", name="ones_bf")
            nc.vector.memset(ones_bf[:], 1.0)
            masku = const.tile([P, P], F8, tag="masku", name="masku")
            nc.sync.dma_start(masku[:], masku_in[:])
            eps_t = const.tile([P, 1], F32, tag="eps", name="eps")
            nc.vector.memset(eps_t[:], EPS)

            wsb = {}
            for k in wcol_in:
                wt = const.tile([P, TT, N], F32, tag=f"w_{k}", name=f"w_{k}")
                nc.sync.dma_start(wt[:], wcol_in[k].rearrange("(tt p) n -> p tt n", p=P))
                wsb[k] = wt

            # ---------- DRAM bounce buffers for collectives ----------
            qk_b = dram.tile([NC * 2 * P, T], F8, tag="qk_b", name="qk_b")
            qk_o = dram.tile([NC * 2 * P, T], F8, tag="qk_o", name="qk_o")
            v_b = dram.tile([BS, P], F8, tag="v_b", name="v_b")
            v_o = dram.tile([BS, P], F8, tag="v_o", name="v_o")
            ab_bs = [dram.tile([NC * P, TB], F8, tag=f"ab_b{b}", name=f"ab_b{b}")
                     for b in range(B)]
            ab_os = [dram.tile([NC * P, TB], F8, tag=f"ab_o{b}", name=f"ab_o{b}")
                     for b in range(B)]

            # ================= stage A: LN1 + transpose + features ========
            with tc.tile_pool(name="stagea", bufs=1) as stagea:
                nxT = {}
                with tc.tile_pool(name="ps_tr", bufs=2, space="PSUM") as ps_tr:
                    for tt in range(TT):
                        nx_t = stagea.tile([P, D], BF, tag="nxa", name=f"nxa{tt}", bufs=2)
                        _layernorm(nc, cpool, x_t[tt], nx_t, eps_t, f"ln1_{tt}")
                        for dcp in range(DCP):
                            pst = ps_tr.tile([P, 2, P], BF, tag="tp", name="tp")
                            for i in range(2):
                                dc = 2 * dcp + i
                                nc.tensor.transpose(pst[:, i], nx_t[:, P * dc:P * (dc + 1)],
                                                    identb[:])
                            t8 = stagea.tile([P, 2, P], F8, tag=f"nxT{tt}_{dcp}",
                                             name=f"nxT{tt}_{dcp}")
                            nc.vector.tensor_copy(t8[:], pst[:])
                            nxT[(tt, dcp)] = t8

                with tc.tile_pool(name="ps_feat", bufs=8, space="PSUM") as ps_feat:
                    for tt in range(TT):
                        psf = [ps_feat.tile([P, 512], F32, tag="feat", name=f"feat{tt}_{g}")
                               for g in range(8)]
                        for dcp in range(DCP):
                            lhs = nxT[(tt, dcp)][:]
                            for g in range(8):
                                src = (fqk_sb if g < 4 else fv_sb)[dcp]
                                gg = g % 4
                                nc.tensor.matmul(
                                    psf[g][:], lhs, src[:, :, 2 * gg:2 * (gg + 1), :],
                                    start=(dcp == 0), stop=(dcp == DCP - 1), perf_mode=DR)
                        for m in range(N):
                            bank, half = m // 2, m % 2
                            pq = psf[bank][:, R * half:R * (half + 1)]
                            pv = psf[4 + bank][:, R * half:R * (half + 1)]
                            for htiles, hbfs, wk, ps_slice in (
                                    (h_q, hbf_q, "wfq", pq), (h_k, hbf_k, "wfk", pq),
                                    (h_v, hbf_v, "wfv", pv)):
                                wcol = wsb[wk][:, tt, m:m + 1]
                                if m == 0:
                                    nc.vector.tensor_scalar_mul(htiles[tt][:], ps_slice, wcol)
                                elif m == N - 1:
                                    nc.vector.scalar_tensor_tensor(
                                        hbfs[tt][:], ps_slice, wcol, htiles[tt][:],
                                        op0=OP.mult, op1=OP.add)
                                else:
                                    nc.vector.scalar_tensor_tensor(
                                        htiles[tt][:], ps_slice, wcol, htiles[tt][:],
                                        op0=OP.mult, op1=OP.add)

            fw_ctx.close()

            if STAGES == 1:
                for tt in range(TT):
                    hq_ev = cpool.tile([P, R], F32, tag="hq_ev", name=f"hq_ev{tt}")
                    nc.vector.tensor_copy(hq_ev[:], h_q[tt][:])
                    nc.sync.dma_start(out_ap[P * tt:P * (tt + 1), 0:R], hq_ev[:])

            # ================= stage B: restores + A2A =====================
            if STAGES >= 2:
                with tc.tile_pool(name="bpool", bufs=1) as bpool, \
                     tc.tile_pool(name="ap_pool", bufs=2) as ap_pool, \
                     tc.tile_pool(name="ps_bt", bufs=2, space="PSUM") as ps_bt, \
                     tc.tile_pool(name="ps_bc", bufs=2, space="PSUM") as ps_bc, \
                     tc.tile_pool(name="ps_r", bufs=4, space="PSUM") as ps_r, \
                     tc.tile_pool(name="ev_b", bufs=3) as ev_b:
                    wbc = {}
                    for k in ["wrqT", "wrkT", "wrvT"]:
                        tiles = []
                        for n in range(N):
                            rowt = bpool.tile([1, T], BF, tag="wrow", name=f"{k}row{n}", bufs=2)
                            nc.sync.dma_start(rowt[:], wrow_in[k][n:n + 1, :])
                            psb = ps_bc.tile([P, T], F32, tag="wbc_ps", name="wbc_ps")
                            nc.tensor.matmul(psb[:], ones_bf[:], rowt[:])
                            wt = bpool.tile([P, T], BF, tag=f"wbc_{k}", name=f"wbc_{k}{n}")
                            nc.scalar.activation(wt[:], psb[:], AF.Copy)
                            tiles.append(wt)
                        wbc[k] = tiles
                    hT = {}
                    for key, hbfs in (("q", hbf_q), ("k", hbf_k), ("v", hbf_v)):
                        ht = bpool.tile([P, RC, T], BF, tag=f"hT{key}", name=f"hT{key}")
                        for tt in range(TT):
                            pst = ps_bt.tile([P, 2, P], BF, tag="tpb", name="tpb")
                            for rc in range(RC):
                                nc.tensor.transpose(pst[:, rc],
                                                    hbfs[tt][:, P * rc:P * (rc + 1)],
                                                    identb[:])
                            nc.vector.tensor_copy(ht[:, :, P * tt:P * (tt + 1)], pst[:])
                        hT[key] = ht

                    def build_A(hkey, wkey):
                        A = [ap_pool.tile([P, RC, T], F8, tag=f"A{n}", name=f"A_{wkey}{n}")
                             for n in range(N)]
                        for n in range(N):
                            for rc in range(RC):
                                nc.vector.tensor_mul(
                                    A[n][:, rc, :], hT[hkey][:, rc, :], wbc[wkey][n][:])
                        return A

                    def qk_restore(A, row_off):
                        for dm in range(DC):
                            ps = ps_r.tile([P, T], F32, tag="r_ps", name="r_ps")
                            for n in range(N):
                                nc.tensor.matmul(
                                    ps[:], rqk_sb[:, n, :, P * dm:P * (dm + 1)], A[n][:],
                                    start=(n == 0), stop=(n == N - 1), perf_mode=DR)
                            ev = ev_b.tile([P, T], F8, tag="ev_qk", name="ev_qk")
                            nc.scalar.activation(ev[:], ps[:], AF.Copy)
                            nc.scalar.dma_start(
                                qk_b[2 * P * dm + row_off: 2 * P * dm + row_off + P, :],
                                ev[:])

                    Aq = build_A("q", "wrqT")
                    Ak = build_A("k", "wrkT")
                    Av = build_A("v", "wrvT")
                    qk_restore(Aq, 0)
                    qk_restore(Ak, P)
                    cc_qk = nc.gpsimd.collective_compute(
                        "AllToAll", OP.bypass, replica_groups=[list(range(NC))],
                        ins=[qk_b.opt()], outs=[qk_o.opt()])
                    first_v_mm = [None]
                    for tt in range(TT):
                        for jf in range(2):
                            ps = ps_r.tile([P, 512], F32, tag="r_ps", name="v_ps")
                            for n in range(N):
                                mm = nc.tensor.matmul(
                                    ps[:], Av[n][:, :, P * tt:P * (tt + 1)],
                                    rv_sb[:, n, :, 512 * jf:512 * (jf + 1)],
                                    start=(n == 0), stop=(n == N - 1), perf_mode=DR)
                                if first_v_mm[0] is None:
                                    first_v_mm[0] = mm
                            ev = ev_b.tile([P, 512], F8, tag="ev_v", name="ev_v")
                            nc.vector.tensor_copy(ev[:], ps[:])
                            for db in range(4):
                                d = 4 * jf + db
                                nc.sync.dma_start(
                                    v_b[T * d + P * tt: T * d + P * (tt + 1), :],
                                    ev[:, P * db:P * (db + 1)])
                    nc.gpsimd.collective_compute(
                        "AllToAll", OP.bypass, replica_groups=[list(range(NC))],
                        ins=[v_b.opt()], outs=[v_o.opt()])
                    # hold V-restore matmuls until the qk collective trigger has
                    # fired, so they execute inside its completion window
                    from concourse.tile import add_dep_helper as _adh
                    try:
                        _adh(cc_qk.ins if hasattr(cc_qk, "ins") else cc_qk,
                             first_v_mm[0].ins if hasattr(first_v_mm[0], "ins")
                             else first_v_mm[0],
                             sync=True, reason="V restore covers qk A2A window")
                    except Exception:
                        pass
                    for n in range(N):
                        rowt = bpool.tile([1, T], BF, tag="wrow", name=f"krrow{n}", bufs=2)
                        nc.sync.dma_start(rowt[:], wrow_in["wkrT"][n:n + 1, :])
                        psb = ps_bc.tile([P, T], F32, tag="wbc_ps", name="wbckr_ps")
                        nc.tensor.matmul(psb[:], ones_bf[:], rowt[:])
                        wt = cprep.tile([P, T], BF, tag=f"wbc_kr{n}", name=f"wbc_kr{n}")
                        nc.scalar.activation(wt[:], psb[:], AF.Copy)
                        wbc_kr.append(wt)

            rp_ctx.close()

            if STAGES == 2:
                for tt in range(TT):
                    qo_ev = cpool.tile([P, T], F8, tag="qo_ev", name=f"qo_ev{tt}")
                    nc.sync.dma_start(qo_ev[:], qk_o[2 * P * tt:2 * P * tt + P, :])
                    qo_f = cpool.tile([P, T], F32, tag="qo_f", name=f"qo_f{tt}")
                    nc.vector.tensor_copy(qo_f[:], qo_ev[:])
                    nc.sync.dma_start(out_ap[P * tt:P * (tt + 1), 0:T], qo_f[:])

            # ========== attention (head-sharded, packed) + W_O + stage C ===
            if STAGES >= 3:
                cpers = ctx.enter_context(tc.tile_pool(name="cpers", bufs=1))
                cscr = ctx.enter_context(tc.tile_pool(name="cscr", bufs=2))
                ps_acc = ctx.enter_context(
                    tc.tile_pool(name="ps_acc", bufs=2, space="PSUM"))
                aTs = [cpers.tile([P, DCP, 2, TB], F8, tag=f"aT{b}", name=f"aT{b}")
                       for b in range(B)]
                x2 = [cpers.tile([P, D], F32, tag=f"x2_{tt}", name=f"x2_{tt}")
                      for tt in range(TT)]

                def wo_block(tt):
                    """W_O for token tile tt (tokens of batch tt//2)."""
                    hb, off = tt // 2, P * (tt % 2)
                    for jf in range(2):
                        ps = ps_acc.tile([P, 512], F32, tag="acc", name=f"wo_ps{tt}")
                        for dcp in range(DCP):
                            nc.tensor.matmul(
                                ps[:], aTs[hb][:, dcp, :, off:off + P],
                                wo_sb[:, dcp, :, 512 * jf:512 * (jf + 1)],
                                start=(dcp == 0), stop=(dcp == DCP - 1), perf_mode=DR)
                        nc.vector.tensor_add(
                            x2[tt][:, 512 * jf:512 * (jf + 1)],
                            x_t[tt][:, 512 * jf:512 * (jf + 1)], ps[:])

                with tc.tile_pool(name="qkv_bh", bufs=1) as qkv_bh, \
                     tc.tile_pool(name="pt_pool", bufs=20) as pt_pool, \
                     tc.tile_pool(name="osb_pool", bufs=4) as osb_pool, \
                     tc.tile_pool(name="ps_st", bufs=4, space="PSUM") as ps_st, \
                     tc.tile_pool(name="ps_o", bufs=2, space="PSUM") as ps_o:
                    # all q/k loads first (vp loads wait on the V collective and
                    # would otherwise block batch-1 q/k behind them in the queue);
                    # fine-grained tiles so early score matmuls start per-chunk
                    qtg, ktc, vpss = {}, {}, []
                    for b in range(B):
                        for qg in range(4):
                            t = qkv_bh.tile([P, 512], F8, tag=f"qtg{b}_{qg}",
                                            name=f"qtg{b}_{qg}")
                            for half in range(2):
                                c = 2 * qg + half
                                nc.scalar.dma_start(
                                    t[:, TB * half:TB * (half + 1)],
                                    qk_o[2 * P * c:2 * P * c + P, TB * b:TB * (b + 1)])
                            qtg[(b, qg)] = t
                            for half in range(2):
                                c = 2 * qg + half
                                kt_t = qkv_bh.tile([P, TB], F8, tag=f"ktc{b}_{c}",
                                                   name=f"ktc{b}_{c}")
                                nc.scalar.dma_start(
                                    kt_t[:], qk_o[2 * P * c + P:2 * P * (c + 1),
                                                  TB * b:TB * (b + 1)])
                                ktc[(b, c)] = kt_t
                    for b in range(B):
                        vps = []
                        for h2 in range(2):
                            vp = qkv_bh.tile([P, S // P // 2, 2, 80], F8,
                                             tag=f"vp{h2}_{b}", name=f"vp{h2}_{b}")
                            src = v_o.rearrange("(c hb i p) f -> hb p c i f",
                                                hb=2, i=2, p=P)[b]
                            for i in range(2):
                                nc.sync.dma_start(
                                    vp[:, :, i, 0:DH],
                                    src[:, :, i, DH * h2:DH * (h2 + 1)])
                            nc.vector.memset(vp[:, :, :, DH:DH + 1], 1.0)
                            vps.append(vp)
                        vpss.append(vps)
                    # stage-C weights: needed only after attention; emitted after
                    # the attention loads so they don't block those DMA queues
                    for dcp in range(DCP):
                        for i in range(2):
                            dc = 2 * dcp + i
                            nc.sync.dma_start(wo_sb[:, dcp, i],
                                              wo_in[P * dc:P * (dc + 1), :])
                            nc.sync.dma_start(
                                fkn_sb[:, dcp, i],
                                fkn_in[:, P * dc:P * (dc + 1), :].rearrange("n p f -> p n f"))
                    for n in range(N):
                        nc.sync.dma_start(rkn_sb[:, n], rkn_in[n])

                    for b in range(B):
                        vps = vpss[b]
                        for qg in range(4):
                            o_ps = [ps_o.tile([DH + 1, 512], F32, tag="o_ps",
                                              name=f"o{b}_{qg}_{h2}") for h2 in range(2)]
                            nkt = 4 * qg + 4
                            for u in range(nkt // 2):
                                pt2 = [pt_pool.tile([P, 2, 512], F8, tag="pt",
                                                    name=f"pt{b}_{qg}_{u}_{h2}")
                                       for h2 in range(2)]
                                for i in range(2):
                                    kt = 2 * u + i
                                    j = kt - 4 * qg
                                    for h2 in range(2):
                                        st = ps_st.tile([P, 512], F32, tag="st", name="st")
                                        koff = P * (kt % 2)
                                        nc.tensor.matmul(
                                            st[:],
                                            ktc[(b, kt // 2)][DH * h2:DH * (h2 + 1),
                                                              koff:koff + P],
                                            qtg[(b, qg)][DH * h2:DH * (h2 + 1), :])
                                        pt = pt2[h2][:, i, :]
                                        if j < 0:
                                            nc.scalar.activation(pt, st[:], AF.Exp,
                                                                 scale=0.125)
                                        else:
                                            if j > 0:
                                                nc.vector.memset(pt[:, 0:P * j], 0.0)
                                            nc.scalar.activation(
                                                pt[:, P * j:], st[:, P * j:],
                                                AF.Exp, scale=0.125)
                                            nc.vector.tensor_mul(
                                                pt[:, P * j:P * (j + 1)],
                                                pt[:, P * j:P * (j + 1)], masku[:])
                                for h2 in range(2):
                                    nc.tensor.matmul(
                                        o_ps[h2][:], vps[h2][:, u, :, 0:DH + 1], pt2[h2][:],
                                        start=(u == 0), stop=(u == nkt // 2 - 1),
                                        perf_mode=DR)
                            for h2 in range(2):
                                # evacuate PSUM immediately; normalize from SBUF
                                den = cpool.tile([1, 512], F32, tag="den", name="den")
                                nc.vector.tensor_copy(den[:], o_ps[h2][DH:DH + 1, :])
                                o_sb = osb_pool.tile([DH, 512], BF, tag="o_sb",
                                                     name=f"osb{b}_{qg}_{h2}")
                                nc.vector.tensor_copy(o_sb[:], o_ps[h2][0:DH, :])
                                rec_f = cpool.tile([1, 512], F32, tag="rec_f", name="rec_f")
                                nc.vector.reciprocal_approx_fast(rec_f[:], den[:])
                                rec = cpool.tile([1, 512], BF, tag="rec", name="rec")
                                nc.vector.tensor_copy(rec[:], rec_f[:])
                                bc = ps_st.tile([DH, 512], F32, tag="st", name="bc")
                                nc.tensor.matmul(bc[:], ones_bf[:, 0:DH], rec[:])
                                bc_sb = cpool.tile([DH, 512], BF, tag="bc_sb", name="bc_sb")
                                nc.vector.tensor_copy(bc_sb[:], bc[:])
                                nrm = cpool.tile([DH, 512], F8, tag="nrm", name="nrm")
                                nc.vector.tensor_mul(nrm[:], o_sb[:], bc_sb[:])
                                for half in range(2):
                                    cblk = 2 * qg + half
                                    nc.sync.dma_start(
                                        ab_bs[b][P * cblk + DH * h2:
                                                 P * cblk + DH * (h2 + 1), :],
                                        nrm[:, TB * half:TB * (half + 1)])
                        nc.gpsimd.collective_compute(
                            "AllToAll", OP.bypass, replica_groups=[list(range(NC))],
                            ins=[ab_bs[b].opt()], outs=[ab_os[b].opt()])
                        for dcp in range(DCP):
                            for i in range(2):
                                dc = 2 * dcp + i
                                nc.scalar.dma_start(aTs[b][:, dcp, i],
                                                  ab_os[b][P * dc:P * (dc + 1), :])
                        if b == 1:
                            # W_O for batch 0 tokens: fills the PE while the
                            # batch-1 A2A completes
                            wo_block(0)
                            wo_block(1)

                if STAGES == 3:
                    for tt in range(TT):
                        at_f = cpool.tile([P, D], F32, tag="at_f", name=f"at_f{tt}")
                        nc.vector.tensor_copy(at_f[:], x2[tt % 2][:])
                        nc.sync.dma_start(out_ap[P * tt:P * (tt + 1), :], at_f[:])

                # ---------------- stage C ----------------
                if STAGES >= 4:
                    with tc.tile_pool(name="ps_ct", bufs=2, space="PSUM") as ps_ct, \
                         tc.tile_pool(name="ps_kf", bufs=2, space="PSUM") as ps_kf, \
                         tc.tile_pool(name="ps_kr", bufs=2, space="PSUM") as ps_kr:
                        wo_block(2)
                        wo_block(3)
                        nx2T = {}
                        hknT = cpers.tile([P, T], BF, tag="hknT", name="hknT")
                        for tt in range(TT):
                            if STAGES >= 5:
                                nx2 = cscr.tile([P, D], BF, tag="nx2", name=f"nx2_{tt}")
                                _layernorm(nc, cpool, x2[tt], nx2, eps_t, f"ln2_{tt}")
                                for dcp in range(DCP):
                                    pst = ps_ct.tile([P, 2, P], BF, tag="tpc", name="tpc")
                                    for i in range(2):
                                        dc = 2 * dcp + i
                                        nc.tensor.transpose(
                                            pst[:, i], nx2[:, P * dc:P * (dc + 1)], identb[:])
                                    t8 = cscr.tile([P, 2, P], F8, tag=f"nx2T{tt}_{dcp}",
                                                   name=f"nx2T{tt}_{dcp}", bufs=1)
                                    nc.vector.tensor_copy(t8[:], pst[:])
                                    nx2T[(tt, dcp)] = t8

                        if STAGES == 4:
                            for tt in range(TT):
                                ao_ev = cpool.tile([P, D], F32, tag="ao_ev", name=f"ao_ev{tt}")
                                nc.vector.tensor_copy(ao_ev[:], x2[tt][:])
                                nc.sync.dma_start(out_ap[P * tt:P * (tt + 1), :], ao_ev[:])

                        if STAGES >= 5:
                            for tt in range(TT):
                                psk = [ps_kf.tile([P, 512], F32, tag="kf", name=f"kf{tt}_{g}")
                                       for g in range(2)]
                                for dcp in range(DCP):
                                    lhs = nx2T[(tt, dcp)][:]
                                    for g in range(2):
                                        nc.tensor.matmul(
                                            psk[g][:], lhs,
                                            fkn_sb[:, dcp, :, 4 * g:4 * (g + 1), :],
                                            start=(dcp == 0), stop=(dcp == DCP - 1),
                                            perf_mode=DR)
                                hkn = cscr.tile([P, KR], F32, tag="hkn", name=f"hkn{tt}")
                                for m in range(N):
                                    pslice = psk[m // 4][:, KR * (m % 4):KR * (m % 4 + 1)]
                                    wcol = wsb["wkf"][:, tt, m:m + 1]
                                    if m == 0:
                                        nc.vector.tensor_scalar_mul(hkn[:], pslice, wcol)
                                    else:
                                        nc.vector.scalar_tensor_tensor(
                                            hkn[:], pslice, wcol, hkn[:],
                                            op0=OP.mult, op1=OP.add)
                                hknb = cscr.tile([P, KR], BF, tag="hknb", name=f"hknb{tt}")
                                nc.vector.tensor_copy(hknb[:], hkn[:])
                                pst = ps_ct.tile([P, P], BF, tag="tpc", name="tpc_kn")
                                nc.tensor.transpose(pst[:], hknb[:], identb[:])
                                nc.vector.tensor_copy(hknT[:, P * tt:P * (tt + 1)], pst[:])

                            Akn = cpers.tile([P, N, T], F8, tag="Akn", name="Akn")
                            for n in range(N):
                                nc.vector.tensor_mul(Akn[:, n, :], hknT[:], wbc_kr[n][:])
                            for tt in range(TT):
                                for jf in range(2):
                                    ps = ps_kr.tile([P, 512], F32, tag="kr_ps", name="kn_ps")
                                    for u in range(N // 2):
                                        nc.tensor.matmul(
                                            ps[:],
                                            Akn[:, 2 * u:2 * (u + 1), P * tt:P * (tt + 1)],
                                            rkn_sb[:, 2 * u:2 * (u + 1),
                                                   512 * jf:512 * (jf + 1)],
                                            start=(u == 0), stop=(u == N // 2 - 1),
                                            perf_mode=DR)
                                    out_sb = cscr.tile([P, 512], F32, tag="out_sb",
                                                       name="out_sb")
                                    nc.vector.tensor_add(
                                        out_sb[:], x2[tt][:, 512 * jf:512 * (jf + 1)], ps[:])
                                    nc.sync.dma_start(
                                        out_ap[P * tt:P * (tt + 1),
                                               512 * jf:512 * (jf + 1)],
                                        out_sb[:])

    nc.compile()
    return nc


_NC = None


def _get_nc():
    global _NC
    if _NC is None:
        _NC = _build()
    return _NC


def _tok_idx(c):
    """Global token indices owned by core c (batch-interleaved)."""
    return np.r_[TB * c:TB * (c + 1), S * 1 * B // 2 + TB * c:2048 + TB * (c + 1)]


def prepare_in_maps(inputs):
    bf = ml_dtypes.bfloat16
    f8 = ml_dtypes.float8_e4m3
    inp = {k: np.ascontiguousarray(np.asarray(v, dtype=np.float32)) for k, v in inputs.items()}
    x_flat = inp["x"].reshape(BS, D)
    wcols = {
        "wfq": inp["fqk_w_Q"].reshape(BS, N), "wfk": inp["fqk_w_K"].reshape(BS, N),
        "wfv": inp["fv_w"].reshape(BS, N), "wkf": inp["feature_know_w"].reshape(BS, N),
    }
    wrows = {
        "wrqT": inp["rqk_w_Q"].reshape(BS, N), "wrkT": inp["rqk_w_K"].reshape(BS, N),
        "wrvT": inp["rv_w"].reshape(BS, N), "wkrT": inp["restore_know_w"].reshape(BS, N),
    }
    g1 = inp["ln1_g"][None, :, None]
    g2 = inp["ln2_g"][None, :, None]
    assert np.abs(inp["ln1_b"]).max() == 0 and np.abs(inp["ln2_b"]).max() == 0, \
        "nonzero LN bias not supported by this build"
    pools = {
        "fqk_p": (inp["f_qk"] * g1).astype(f8), "fv_p": (inp["f_v"] * g1).astype(f8),
        "rqk_p": inp["r_qk"].astype(f8), "rv_p": inp["r_v"].astype(f8),
        "fkn_p": (inp["f_know"] * g2).astype(f8), "rkn_p": inp["r_know"].astype(f8),
    }
    wo_p = np.ascontiguousarray(inp["W_O"].T).astype(f8)
    masku = np.ascontiguousarray(np.tril(np.ones((P, P), np.float32)).T).astype(f8)

    in_maps = []
    for c in range(NC):
        idx = np.r_[TB * c:TB * (c + 1), S + TB * c:S + TB * (c + 1)]
        m = {
            "x_sh": np.ascontiguousarray(x_flat[idx]),
            "wo_p": wo_p, "masku": masku,
        }
        m.update(pools)
        for k, v in wcols.items():
            m[k] = np.ascontiguousarray(v[idx])
        for k, v in wrows.items():
            m[k] = np.ascontiguousarray(v[idx].T).astype(bf)
        in_maps.append(m)
    return in_maps


def kernel(**inputs):
    nc = _get_nc()
    in_maps = prepare_in_maps(inputs)
    res = run_bass_kernel_spmd(nc, in_maps, list(range(NC))).results
    out = np.zeros((BS, D), np.float32)
    for c in range(NC):
        out[TB * c:TB * (c + 1)] = res[c]["out_sh"][0:TB]
        out[S + TB * c:S + TB * (c + 1)] = res[c]["out_sh"][TB:T]
    return out.reshape(B, S, D)


# revision 27
# speedup vs baseline: 1.0083x; 1.0083x over previous
"""Trainium2 Bass kernel for nn_DAWNBlock (8-core SPMD), v5.

Decomposition (validated in numpy: proto_check.py, quant_check.py):
  - Token-sharded with batch-interleaved ownership: core c owns global tokens
    [256c, 256c+256) of batch 0 AND [256c, 256c+256) of batch 1 (T=512).
    LN1, features, restores (Q/K/V), LN2 + knowledge run token-local.
  - Head-sharded attention: core c owns d-cols [128c, 128c+128) (heads
    {2c, 2c+1}). One A2A reshards Q^T+K^T (1MB fp8), one reshards V (0.5MB).
    Attention output returns via TWO 0.25MB A2As (one per batch) so batch 0's
    resharding and its W_O hide under batch 1's attention.
  - fp8e4m3 + DoubleRow (K=256 pairs, 2x PE rate) for features, restores,
    PV, W_O, knowledge. Scores fp8 K=64 packed as concurrent row-group pairs.
    All-fp8 quantization study: 4e-3 rel err vs the 2e-2 gate.
  - LN gains folded into the f-pools host-side (bias asserted zero); LN is
    stats + one ACT apply. Routing weights: feature PSUM banks combined via
    DVE scalar_tensor_tensor; restores use A[n] = hT * wbc[n] (PE-broadcast
    rows), h transposed once.
  - Causal softmax without max-subtraction; denominator via ones-column in V;
    o_ps evacuated to SBUF immediately to release PSUM banks.
"""
import sys

sys.path.insert(0, "/opt/trn_rl_repo")

import os
import numpy as np
import ml_dtypes
import concourse.bass as bass
import concourse.mybir as mybir
import concourse.tile as tile
from concourse import bacc
from concourse.bass_utils import run_bass_kernel_spmd
from concourse.masks import make_identity

B, S, D, H, R, N, KR = 2, 2048, 1024, 16, 256, 8, 128
DH = D // H           # 64
BS = B * S            # 4096
NC = 8
T = BS // NC          # 512 tokens per core (256 per batch)
TB = T // B           # 256 tokens per (core, batch)
P = 128
TT = T // P           # 4 token tiles per core
DC = D // P           # 8 d chunks
DCP = DC // 2         # 4 d chunk-pairs (DoubleRow)
RC = R // P           # 2 r chunks
EPS = 1e-5

STAGES = int(os.environ.get("BASS_STAGES", "5"))
F32 = mybir.dt.float32
BF = mybir.dt.bfloat16
F8 = mybir.dt.float8e4
DR = mybir.MatmulPerfMode.DoubleRow
AF = mybir.ActivationFunctionType
OP = mybir.AluOpType


def _layernorm(nc, cpool, x_sb, nx_sb, eps_tile, tag):
    """nx = (x - mean(x)) * rsqrt(var + eps) for one [128, D] tile.
    (LN gain folded into downstream pools host-side; bias asserted zero.)"""
    s = cpool.tile([P, 1], F32, tag="ln_s", name=f"{tag}_s")
    nm = cpool.tile([P, 1], F32, tag="ln_nm", name=f"{tag}_nm")
    sq = cpool.tile([P, D], F32, tag="ln_sq", name=f"{tag}_sq")
    ssq = cpool.tile([P, 1], F32, tag="ln_ssq", name=f"{tag}_ssq")
    rs = cpool.tile([P, 1], F32, tag="ln_rs", name=f"{tag}_rs")
    nmrs = cpool.tile([P, 1], F32, tag="ln_nmrs", name=f"{tag}_nmrs")
    nc.vector.reduce_sum(s[:], x_sb[:], axis=mybir.AxisListType.X)
    nc.vector.tensor_scalar_mul(nm[:], s[:], -1.0 / D)
    nc.scalar.activation(sq[:], x_sb[:], AF.Square, bias=nm[:], accum_out=ssq[:])
    nc.scalar.activation(rs[:], ssq[:], AF.Abs_reciprocal_sqrt,
                         bias=eps_tile[:], scale=1.0 / D)
    nc.vector.tensor_mul(nmrs[:], nm[:], rs[:])
    nc.scalar.activation(nx_sb[:], x_sb[:], AF.Identity, bias=nmrs[:], scale=rs[:])


def _build():
    nc = bacc.Bacc("TRN2", target_bir_lowering=False, debug=False, num_devices=NC)

    def di(name, shape, dt=F8):
        return nc.dram_tensor(name, shape, dt, kind="ExternalInput").ap()

    x_in = di("x_sh", [T, D], F32)
    wcol_in = {k: di(k, [T, N], F32) for k in ["wfq", "wfk", "wfv", "wkf"]}
    wrow_in = {k: di(k, [N, T], BF) for k in ["wrqT", "wrkT", "wrvT", "wkrT"]}
    fqk_in = di("fqk_p", [N, D, R])
    fv_in = di("fv_p", [N, D, R])
    rqk_in = di("rqk_p", [N, R, D])
    rv_in = di("rv_p", [N, R, D])
    fkn_in = di("fkn_p", [N, D, KR])
    rkn_in = di("rkn_p", [N, KR, D])
    wo_in = di("wo_p", [D, D])       # = W_O.T
    masku_in = di("masku", [P, P])
    out_ap = nc.dram_tensor("out_sh", [T, D], F32, kind="ExternalOutput").ap()

    with tile.TileContext(nc) as tc:
        from contextlib import ExitStack
        with ExitStack() as ctx:
            const = ctx.enter_context(tc.tile_pool(name="const", bufs=1))
            cpool = ctx.enter_context(tc.tile_pool(name="scratch", bufs=2))
            dram = ctx.enter_context(tc.tile_pool(name="dram", bufs=1, space="DRAM"))

            # ---------- pools (stack order: long-lived first) ----------
            xpool = ctx.enter_context(tc.tile_pool(name="xpool", bufs=1))
            x_t = [xpool.tile([P, D], F32, tag=f"x{tt}", name=f"x{tt}") for tt in range(TT)]

            cprep = ctx.enter_context(tc.tile_pool(name="cprep", bufs=1))
            wo_sb = cprep.tile([P, DCP, 2, D], F8, tag="wo_sb", name="wo_sb")
            fkn_sb = cprep.tile([P, DCP, 2, N, KR], F8, tag="fkn_sb", name="fkn_sb")
            rkn_sb = cprep.tile([P, N, D], F8, tag="rkn_sb", name="rkn_sb")
            wbc_kr = []

            hpool = ctx.enter_context(tc.tile_pool(name="hpool", bufs=1))
            h_q = [hpool.tile([P, R], F32, tag=f"hq{tt}", name=f"hq{tt}") for tt in range(TT)]
            h_k = [hpool.tile([P, R], F32, tag=f"hk{tt}", name=f"hk{tt}") for tt in range(TT)]
            h_v = [hpool.tile([P, R], F32, tag=f"hv{tt}", name=f"hv{tt}") for tt in range(TT)]
            hbf_q = [hpool.tile([P, R], BF, tag=f"hbq{tt}", name=f"hbq{tt}") for tt in range(TT)]
            hbf_k = [hpool.tile([P, R], BF, tag=f"hbk{tt}", name=f"hbk{tt}") for tt in range(TT)]
            hbf_v = [hpool.tile([P, R], BF, tag=f"hbv{tt}", name=f"hbv{tt}") for tt in range(TT)]

            from contextlib import ExitStack as _ES
            rp_ctx = _ES()
            rp = rp_ctx.enter_context(tc.tile_pool(name="rp", bufs=1))
            rqk_sb = rp.tile([P, N, RC, D], F8, tag="rqk_sb", name="rqk_sb")
            rv_sb = rp.tile([P, N, RC, D], F8, tag="rv_sb", name="rv_sb")

            fw_ctx = _ES()
            fwpool = fw_ctx.enter_context(tc.tile_pool(name="fwpool", bufs=1))
            fqk_sb = [fwpool.tile([P, 2, N, R], F8, tag=f"fqk{dcp}", name=f"fqk{dcp}")
                      for dcp in range(DCP)]
            fv_sb = [fwpool.tile([P, 2, N, R], F8, tag=f"fv{dcp}", name=f"fv{dcp}")
                     for dcp in range(DCP)]

            # ---------- big prefetches in priority order ----------
            for tt in range(TT):
                nc.sync.dma_start(x_t[tt][:], x_in[P * tt:P * (tt + 1), :])
            for dcp in range(DCP):
                for i in range(2):
                    dc = 2 * dcp + i
                    nc.sync.dma_start(
                        fqk_sb[dcp][:, i],
                        fqk_in[:, P * dc:P * (dc + 1), :].rearrange("n p r -> p n r"))
                    nc.sync.dma_start(
                        fv_sb[dcp][:, i],
                        fv_in[:, P * dc:P * (dc + 1), :].rearrange("n p r -> p n r"))
            for n in range(N):
                nc.sync.dma_start(
                    rqk_sb[:, n], rqk_in[n].rearrange("(rc p) d -> p rc d", p=P))
                nc.sync.dma_start(
                    rv_sb[:, n], rv_in[n].rearrange("(rc p) d -> p rc d", p=P))
            # tiny warm-up collective: pays the first-collective ncfw latency
            dumm_b = dram.tile([NC * 8, 8], F32, tag="dumm_b", name="dumm_b")
            dumm_o = dram.tile([NC * 8, 8], F32, tag="dumm_o", name="dumm_o")
            nc.gpsimd.collective_compute(
                "AllToAll", OP.bypass, replica_groups=[list(range(NC))],
                ins=[dumm_b.opt()], outs=[dumm_o.opt()])

            ident_f = const.tile([P, P], F32, tag="ident_f", name="ident_f")
            make_identity(nc, ident_f)
            identb = const.tile([P, P], BF, tag="identb", name="identb")
            nc.vector.tensor_copy(identb[:], ident_f[:])
            ones_bf = const.tile([1, P], BF, tag="ones_bf", name="ones_bf")
            nc.vector.memset(ones_bf[:], 1.0)
            masku = const.tile([P, P], F8, tag="masku", name="masku")
            nc.sync.dma_start(masku[:], masku_in[:])
            eps_t = const.tile([P, 1], F32, tag="eps", name="eps")
            nc.vector.memset(eps_t[:], EPS)

            wsb = {}
            for k in wcol_in:
                wt = const.tile([P, TT, N], F32, tag=f"w_{k}", name=f"w_{k}")
                nc.sync.dma_start(wt[:], wcol_in[k].rearrange("(tt p) n -> p tt n", p=P))
                wsb[k] = wt

            # ---------- DRAM bounce buffers for collectives ----------
            qk_b = dram.tile([NC * 2 * P, T], F8, tag="qk_b", name="qk_b")
            qk_o = dram.tile([NC * 2 * P, T], F8, tag="qk_o", name="qk_o")
            v_b = dram.tile([BS, P], F8, tag="v_b", name="v_b")
            v_o = dram.tile([BS, P], F8, tag="v_o", name="v_o")
            ab_bs = [dram.tile([NC * P, TB], F8, tag=f"ab_b{b}", name=f"ab_b{b}")
                     for b in range(B)]
            ab_os = [dram.tile([NC * P, TB], F8, tag=f"ab_o{b}", name=f"ab_o{b}")
                     for b in range(B)]

            # ================= stage A: LN1 + transpose + features ========
            with tc.tile_pool(name="stagea", bufs=1) as stagea:
                nxT = {}
                with tc.tile_pool(name="ps_tr", bufs=2, space="PSUM") as ps_tr:
                    for tt in range(TT):
                        nx_t = stagea.tile([P, D], BF, tag="nxa", name=f"nxa{tt}", bufs=2)
                        _layernorm(nc, cpool, x_t[tt], nx_t, eps_t, f"ln1_{tt}")
                        for dcp in range(DCP):
                            pst = ps_tr.tile([P, 2, P], BF, tag="tp", name="tp")
                            for i in range(2):
                                dc = 2 * dcp + i
                                nc.tensor.transpose(pst[:, i], nx_t[:, P * dc:P * (dc + 1)],
                                                    identb[:])
                            t8 = stagea.tile([P, 2, P], F8, tag=f"nxT{tt}_{dcp}",
                                             name=f"nxT{tt}_{dcp}")
                            nc.vector.tensor_copy(t8[:], pst[:])
                            nxT[(tt, dcp)] = t8

                with tc.tile_pool(name="ps_feat", bufs=8, space="PSUM") as ps_feat:
                    for tt in range(TT):
                        psf = [ps_feat.tile([P, 512], F32, tag="feat", name=f"feat{tt}_{g}")
                               for g in range(8)]
                        for dcp in range(DCP):
                            lhs = nxT[(tt, dcp)][:]
                            for g in range(8):
                                src = (fqk_sb if g < 4 else fv_sb)[dcp]
                                gg = g % 4
                                nc.tensor.matmul(
                                    psf[g][:], lhs, src[:, :, 2 * gg:2 * (gg + 1), :],
                                    start=(dcp == 0), stop=(dcp == DCP - 1), perf_mode=DR)
                        for m in range(N):
                            bank, half = m // 2, m % 2
                            pq = psf[bank][:, R * half:R * (half + 1)]
                            pv = psf[4 + bank][:, R * half:R * (half + 1)]
                            for htiles, hbfs, wk, ps_slice in (
                                    (h_q, hbf_q, "wfq", pq), (h_k, hbf_k, "wfk", pq),
                                    (h_v, hbf_v, "wfv", pv)):
                                wcol = wsb[wk][:, tt, m:m + 1]
                                if m == 0:
                                    nc.vector.tensor_scalar_mul(htiles[tt][:], ps_slice, wcol)
                                elif m == N - 1:
                                    nc.vector.scalar_tensor_tensor(
                                        hbfs[tt][:], ps_slice, wcol, htiles[tt][:],
                                        op0=OP.mult, op1=OP.add)
                                else:
                                    nc.vector.scalar_tensor_tensor(
                                        htiles[tt][:], ps_slice, wcol, htiles[tt][:],
                                        op0=OP.mult, op1=OP.add)

            fw_ctx.close()

            if STAGES == 1:
                for tt in range(TT):
                    hq_ev = cpool.tile([P, R], F32, tag="hq_ev", name=f"hq_ev{tt}")
                    nc.vector.tensor_copy(hq_ev[:], h_q[tt][:])
                    nc.sync.dma_start(out_ap[P * tt:P * (tt + 1), 0:R], hq_ev[:])

            # ================= stage B: restores + A2A =====================
            if STAGES >= 2:
                with tc.tile_pool(name="bpool", bufs=1) as bpool, \
                     tc.tile_pool(name="ap_pool", bufs=2) as ap_pool, \
                     tc.tile_pool(name="ps_bt", bufs=2, space="PSUM") as ps_bt, \
                     tc.tile_pool(name="ps_bc", bufs=2, space="PSUM") as ps_bc, \
                     tc.tile_pool(name="ps_r", bufs=4, space="PSUM") as ps_r, \
                     tc.tile_pool(name="ev_b", bufs=3) as ev_b:
                    wbc = {}
                    for k in ["wrqT", "wrkT", "wrvT"]:
                        tiles = []
                        for n in range(N):
                            rowt = bpool.tile([1, T], BF, tag="wrow", name=f"{k}row{n}", bufs=2)
                            nc.sync.dma_start(rowt[:], wrow_in[k][n:n + 1, :])
                            psb = ps_bc.tile([P, T], F32, tag="wbc_ps", name="wbc_ps")
                            nc.tensor.matmul(psb[:], ones_bf[:], rowt[:])
                            wt = bpool.tile([P, T], BF, tag=f"wbc_{k}", name=f"wbc_{k}{n}")
                            nc.scalar.activation(wt[:], psb[:], AF.Copy)
                            tiles.append(wt)
                        wbc[k] = tiles
                    hT = {}
                    for key, hbfs in (("q", hbf_q), ("k", hbf_k), ("v", hbf_v)):
                        ht = bpool.tile([P, RC, T], BF, tag=f"hT{key}", name=f"hT{key}")
                        for tt in range(TT):
                            pst = ps_bt.tile([P, 2, P], BF, tag="tpb", name="tpb")
                            for rc in range(RC):
                                nc.tensor.transpose(pst[:, rc],
                                                    hbfs[tt][:, P * rc:P * (rc + 1)],
                                                    identb[:])
                            nc.vector.tensor_copy(ht[:, :, P * tt:P * (tt + 1)], pst[:])
                        hT[key] = ht

                    def build_A(hkey, wkey):
                        A = [ap_pool.tile([P, RC, T], F8, tag=f"A{n}", name=f"A_{wkey}{n}")
                             for n in range(N)]
                        for n in range(N):
                            for rc in range(RC):
                                nc.vector.tensor_mul(
                                    A[n][:, rc, :], hT[hkey][:, rc, :], wbc[wkey][n][:])
                        return A

                    def qk_restore(A, row_off):
                        for dm in range(DC):
                            ps = ps_r.tile([P, T], F32, tag="r_ps", name="r_ps")
                            for n in range(N):
                                nc.tensor.matmul(
                                    ps[:], rqk_sb[:, n, :, P * dm:P * (dm + 1)], A[n][:],
                                    start=(n == 0), stop=(n == N - 1), perf_mode=DR)
                            ev = ev_b.tile([P, T], F8, tag="ev_qk", name="ev_qk")
                            nc.scalar.activation(ev[:], ps[:], AF.Copy)
                            nc.sync.dma_start(
                                qk_b[2 * P * dm + row_off: 2 * P * dm + row_off + P, :],
                                ev[:])

                    Aq = build_A("q", "wrqT")
                    Ak = build_A("k", "wrkT")
                    Av = build_A("v", "wrvT")
                    qk_restore(Aq, 0)
                    qk_restore(Ak, P)
                    cc_qk = nc.gpsimd.collective_compute(
                        "AllToAll", OP.bypass, replica_groups=[list(range(NC))],
                        ins=[qk_b.opt()], outs=[qk_o.opt()])
                    first_v_mm = [None]
                    for tt in range(TT):
                        for jf in range(2):
                            ps = ps_r.tile([P, 512], F32, tag="r_ps", name="v_ps")
                            for n in range(N):
                                mm = nc.tensor.matmul(
                                    ps[:], Av[n][:, :, P * tt:P * (tt + 1)],
                                    rv_sb[:, n, :, 512 * jf:512 * (jf + 1)],
                                    start=(n == 0), stop=(n == N - 1), perf_mode=DR)
                                if first_v_mm[0] is None:
                                    first_v_mm[0] = mm
                            ev = ev_b.tile([P, 512], F8, tag="ev_v", name="ev_v")
                            nc.vector.tensor_copy(ev[:], ps[:])
                            for db in range(4):
                                d = 4 * jf + db
                                nc.sync.dma_start(
                                    v_b[T * d + P * tt: T * d + P * (tt + 1), :],
                                    ev[:, P * db:P * (db + 1)])
                    nc.gpsimd.collective_compute(
                        "AllToAll", OP.bypass, replica_groups=[list(range(NC))],
                        ins=[v_b.opt()], outs=[v_o.opt()])
                    # hold V-restore matmuls until the qk collective trigger has
                    # fired, so they execute inside its completion window
                    from concourse.tile import add_dep_helper as _adh
                    try:
                        _adh(cc_qk.ins if hasattr(cc_qk, "ins") else cc_qk,
                             first_v_mm[0].ins if hasattr(first_v_mm[0], "ins")
                             else first_v_mm[0],
                             sync=True, reason="V restore covers qk A2A window")
                    except Exception:
                        pass
                    for n in range(N):
                        rowt = bpool.tile([1, T], BF, tag="wrow", name=f"krrow{n}", bufs=2)
                        nc.sync.dma_start(rowt[:], wrow_in["wkrT"][n:n + 1, :])
                        psb = ps_bc.tile([P, T], F32, tag="wbc_ps", name="wbckr_ps")
                        nc.tensor.matmul(psb[:], ones_bf[:], rowt[:])
                        wt = cprep.tile([P, T], BF, tag=f"wbc_kr{n}", name=f"wbc_kr{n}")
                        nc.scalar.activation(wt[:], psb[:], AF.Copy)
                        wbc_kr.append(wt)

            rp_ctx.close()

            if STAGES == 2:
                for tt in range(TT):
                    qo_ev = cpool.tile([P, T], F8, tag="qo_ev", name=f"qo_ev{tt}")
                    nc.sync.dma_start(qo_ev[:], qk_o[2 * P * tt:2 * P * tt + P, :])
                    qo_f = cpool.tile([P, T], F32, tag="qo_f", name=f"qo_f{tt}")
                    nc.vector.tensor_copy(qo_f[:], qo_ev[:])
                    nc.sync.dma_start(out_ap[P * tt:P * (tt + 1), 0:T], qo_f[:])

            # ========== attention (head-sharded, packed) + W_O + stage C ===
            if STAGES >= 3:
                cpers = ctx.enter_context(tc.tile_pool(name="cpers", bufs=1))
                cscr = ctx.enter_context(tc.tile_pool(name="cscr", bufs=2))
                ps_acc = ctx.enter_context(
                    tc.tile_pool(name="ps_acc", bufs=2, space="PSUM"))
                aTs = [cpers.tile([P, DCP, 2, TB], F8, tag=f"aT{b}", name=f"aT{b}")
                       for b in range(B)]
                x2 = [cpers.tile([P, D], F32, tag=f"x2_{tt}", name=f"x2_{tt}")
                      for tt in range(TT)]

                def wo_block(tt):
                    """W_O for token tile tt (tokens of batch tt//2)."""
                    hb, off = tt // 2, P * (tt % 2)
                    for jf in range(2):
                        ps = ps_acc.tile([P, 512], F32, tag="acc", name=f"wo_ps{tt}")
                        for dcp in range(DCP):
                            nc.tensor.matmul(
                                ps[:], aTs[hb][:, dcp, :, off:off + P],
                                wo_sb[:, dcp, :, 512 * jf:512 * (jf + 1)],
                                start=(dcp == 0), stop=(dcp == DCP - 1), perf_mode=DR)
                        nc.vector.tensor_add(
                            x2[tt][:, 512 * jf:512 * (jf + 1)],
                            x_t[tt][:, 512 * jf:512 * (jf + 1)], ps[:])

                with tc.tile_pool(name="qkv_bh", bufs=1) as qkv_bh, \
                     tc.tile_pool(name="pt_pool", bufs=20) as pt_pool, \
                     tc.tile_pool(name="osb_pool", bufs=4) as osb_pool, \
                     tc.tile_pool(name="ps_st", bufs=4, space="PSUM") as ps_st, \
                     tc.tile_pool(name="ps_o", bufs=2, space="PSUM") as ps_o:
                    # all q/k loads first (vp loads wait on the V collective and
                    # would otherwise block batch-1 q/k behind them in the queue);
                    # fine-grained tiles so early score matmuls start per-chunk
                    qtg, ktc, vpss = {}, {}, []
                    for b in range(B):
                        for qg in range(4):
                            t = qkv_bh.tile([P, 512], F8, tag=f"qtg{b}_{qg}",
                                            name=f"qtg{b}_{qg}")
                            for half in range(2):
                                c = 2 * qg + half
                                nc.sync.dma_start(
                                    t[:, TB * half:TB * (half + 1)],
                                    qk_o[2 * P * c:2 * P * c + P, TB * b:TB * (b + 1)])
                            qtg[(b, qg)] = t
                        for c in range(NC):
                            kt_t = qkv_bh.tile([P, TB], F8, tag=f"ktc{b}_{c}",
                                               name=f"ktc{b}_{c}")
                            nc.sync.dma_start(
                                kt_t[:], qk_o[2 * P * c + P:2 * P * (c + 1),
                                              TB * b:TB * (b + 1)])
                            ktc[(b, c)] = kt_t
                    for b in range(B):
                        vps = []
                        for h2 in range(2):
                            vp = qkv_bh.tile([P, S // P // 2, 2, 80], F8,
                                             tag=f"vp{h2}_{b}", name=f"vp{h2}_{b}")
                            src = v_o.rearrange("(c hb i p) f -> hb p c i f",
                                                hb=2, i=2, p=P)[b]
                            for i in range(2):
                                nc.sync.dma_start(
                                    vp[:, :, i, 0:DH],
                                    src[:, :, i, DH * h2:DH * (h2 + 1)])
                            nc.vector.memset(vp[:, :, :, DH:DH + 1], 1.0)
                            vps.append(vp)
                        vpss.append(vps)
                    # stage-C weights: needed only after attention; emitted after
                    # the attention loads so they don't block those DMA queues
                    for dcp in range(DCP):
                        for i in range(2):
                            dc = 2 * dcp + i
                            nc.sync.dma_start(wo_sb[:, dcp, i],
                                              wo_in[P * dc:P * (dc + 1), :])
                            nc.sync.dma_start(
                                fkn_sb[:, dcp, i],
                                fkn_in[:, P * dc:P * (dc + 1), :].rearrange("n p f -> p n f"))
                    for n in range(N):
                        nc.sync.dma_start(rkn_sb[:, n], rkn_in[n])

                    for b in range(B):
                        vps = vpss[b]
                        for qg in range(4):
                            o_ps = [ps_o.tile([DH + 1, 512], F32, tag="o_ps",
                                              name=f"o{b}_{qg}_{h2}") for h2 in range(2)]
                            nkt = 4 * qg + 4
                            for u in range(nkt // 2):
                                pt2 = [pt_pool.tile([P, 2, 512], F8, tag="pt",
                                                    name=f"pt{b}_{qg}_{u}_{h2}")
                                       for h2 in range(2)]
                                for i in range(2):
                                    kt = 2 * u + i
                                    j = kt - 4 * qg
                                    for h2 in range(2):
                                        st = ps_st.tile([P, 512], F32, tag="st", name="st")
                                        koff = P * (kt % 2)
                                        nc.tensor.matmul(
                                            st[:],
                                            ktc[(b, kt // 2)][DH * h2:DH * (h2 + 1),
                                                              koff:koff + P],
                                            qtg[(b, qg)][DH * h2:DH * (h2 + 1), :])
                                        pt = pt2[h2][:, i, :]
                                        if j < 0:
                                            nc.scalar.activation(pt, st[:], AF.Exp,
                                                                 scale=0.125)
                                        else:
                                            if j > 0:
                                                nc.vector.memset(pt[:, 0:P * j], 0.0)
                                            nc.scalar.activation(
                                                pt[:, P * j:], st[:, P * j:],
                                                AF.Exp, scale=0.125)
                                            nc.vector.tensor_mul(
                                                pt[:, P * j:P * (j + 1)],
                                                pt[:, P * j:P * (j + 1)], masku[:])
                                for h2 in range(2):
                                    nc.tensor.matmul(
                                        o_ps[h2][:], vps[h2][:, u, :, 0:DH + 1], pt2[h2][:],
                                        start=(u == 0), stop=(u == nkt // 2 - 1),
                                        perf_mode=DR)
                            for h2 in range(2):
                                # evacuate PSUM immediately; normalize from SBUF
                                den = cpool.tile([1, 512], F32, tag="den", name="den")
                                nc.vector.tensor_copy(den[:], o_ps[h2][DH:DH + 1, :])
                                o_sb = osb_pool.tile([DH, 512], BF, tag="o_sb",
                                                     name=f"osb{b}_{qg}_{h2}")
                                nc.vector.tensor_copy(o_sb[:], o_ps[h2][0:DH, :])
                                rec_f = cpool.tile([1, 512], F32, tag="rec_f", name="rec_f")
                                nc.vector.reciprocal_approx_fast(rec_f[:], den[:])
                                rec = cpool.tile([1, 512], BF, tag="rec", name="rec")
                                nc.vector.tensor_copy(rec[:], rec_f[:])
                                bc = ps_st.tile([DH, 512], F32, tag="st", name="bc")
                                nc.tensor.matmul(bc[:], ones_bf[:, 0:DH], rec[:])
                                bc_sb = cpool.tile([DH, 512], BF, tag="bc_sb", name="bc_sb")
                                nc.vector.tensor_copy(bc_sb[:], bc[:])
                                nrm = cpool.tile([DH, 512], F8, tag="nrm", name="nrm")
                                nc.vector.tensor_mul(nrm[:], o_sb[:], bc_sb[:])
                                for half in range(2):
                                    cblk = 2 * qg + half
                                    nc.sync.dma_start(
                                        ab_bs[b][P * cblk + DH * h2:
                                                 P * cblk + DH * (h2 + 1), :],
                                        nrm[:, TB * half:TB * (half + 1)])
                        nc.gpsimd.collective_compute(
                            "AllToAll", OP.bypass, replica_groups=[list(range(NC))],
                            ins=[ab_bs[b].opt()], outs=[ab_os[b].opt()])
                        for dcp in range(DCP):
                            for i in range(2):
                                dc = 2 * dcp + i
                                nc.sync.dma_start(aTs[b][:, dcp, i],
                                                  ab_os[b][P * dc:P * (dc + 1), :])
                        if b == 1:
                            # W_O for batch 0 tokens: fills the PE while the
                            # batch-1 A2A completes
                            wo_block(0)
                            wo_block(1)

                if STAGES == 3:
                    for tt in range(TT):
                        at_f = cpool.tile([P, D], F32, tag="at_f", name=f"at_f{tt}")
                        nc.vector.tensor_copy(at_f[:], x2[tt % 2][:])
                        nc.sync.dma_start(out_ap[P * tt:P * (tt + 1), :], at_f[:])

                # ---------------- stage C ----------------
                if STAGES >= 4:
                    with tc.tile_pool(name="ps_ct", bufs=2, space="PSUM") as ps_ct, \
                         tc.tile_pool(name="ps_kf", bufs=2, space="PSUM") as ps_kf, \
                         tc.tile_pool(name="ps_kr", bufs=2, space="PSUM") as ps_kr:
                        wo_block(2)
                        wo_block(3)
                        nx2T = {}
                        hknT = cpers.tile([P, T], BF, tag="hknT", name="hknT")
                        for tt in range(TT):
                            if STAGES >= 5:
                                nx2 = cscr.tile([P, D], BF, tag="nx2", name=f"nx2_{tt}")
                                _layernorm(nc, cpool, x2[tt], nx2, eps_t, f"ln2_{tt}")
                                for dcp in range(DCP):
                                    pst = ps_ct.tile([P, 2, P], BF, tag="tpc", name="tpc")
                                    for i in range(2):
                                        dc = 2 * dcp + i
                                        nc.tensor.transpose(
                                            pst[:, i], nx2[:, P * dc:P * (dc + 1)], identb[:])
                                    t8 = cscr.tile([P, 2, P], F8, tag=f"nx2T{tt}_{dcp}",
                                                   name=f"nx2T{tt}_{dcp}", bufs=1)
                                    nc.vector.tensor_copy(t8[:], pst[:])
                                    nx2T[(tt, dcp)] = t8

                        if STAGES == 4:
                            for tt in range(TT):
                                ao_ev = cpool.tile([P, D], F32, tag="ao_ev", name=f"ao_ev{tt}")
                                nc.vector.tensor_copy(ao_ev[:], x2[tt][:])
                                nc.sync.dma_start(out_ap[P * tt:P * (tt + 1), :], ao_ev[:])

                        if STAGES >= 5:
                            for tt in range(TT):
                                psk = [ps_kf.tile([P, 512], F32, tag="kf", name=f"kf{tt}_{g}")
                                       for g in range(2)]
                                for dcp in range(DCP):
                                    lhs = nx2T[(tt, dcp)][:]
                                    for g in range(2):
                                        nc.tensor.matmul(
                                            psk[g][:], lhs,
                                            fkn_sb[:, dcp, :, 4 * g:4 * (g + 1), :],
                                            start=(dcp == 0), stop=(dcp == DCP - 1),
                                            perf_mode=DR)
                                hkn = cscr.tile([P, KR], F32, tag="hkn", name=f"hkn{tt}")
                                for m in range(N):
                                    pslice = psk[m // 4][:, KR * (m % 4):KR * (m % 4 + 1)]
                                    wcol = wsb["wkf"][:, tt, m:m + 1]
                                    if m == 0:
                                        nc.vector.tensor_scalar_mul(hkn[:], pslice, wcol)
                                    else:
                                        nc.vector.scalar_tensor_tensor(
                                            hkn[:], pslice, wcol, hkn[:],
                                            op0=OP.mult, op1=OP.add)
                                hknb = cscr.tile([P, KR], BF, tag="hknb", name=f"hknb{tt}")
                                nc.vector.tensor_copy(hknb[:], hkn[:])
                                pst = ps_ct.tile([P, P], BF, tag="tpc", name="tpc_kn")
                                nc.tensor.transpose(pst[:], hknb[:], identb[:])
                                nc.vector.tensor_copy(hknT[:, P * tt:P * (tt + 1)], pst[:])

                            Akn = cpers.tile([P, N, T], F8, tag="Akn", name="Akn")
                            for n in range(N):
                                nc.vector.tensor_mul(Akn[:, n, :], hknT[:], wbc_kr[n][:])
                            for tt in range(TT):
                                for jf in range(2):
                                    ps = ps_kr.tile([P, 512], F32, tag="kr_ps", name="kn_ps")
                                    for u in range(N // 2):
                                        nc.tensor.matmul(
                                            ps[:],
                                            Akn[:, 2 * u:2 * (u + 1), P * tt:P * (tt + 1)],
                                            rkn_sb[:, 2 * u:2 * (u + 1),
                                                   512 * jf:512 * (jf + 1)],
                                            start=(u == 0), stop=(u == N // 2 - 1),
                                            perf_mode=DR)
                                    out_sb = cscr.tile([P, 512], F32, tag="out_sb",
                                                       name="out_sb")
                                    nc.vector.tensor_add(
                                        out_sb[:], x2[tt][:, 512 * jf:512 * (jf + 1)], ps[:])
                                    nc.sync.dma_start(
                                        out_ap[P * tt:P * (tt + 1),
                                               512 * jf:512 * (jf + 1)],
                                        out_sb[:])

    nc.compile()
    return nc


_NC = None


def _get_nc():
    global _NC
    if _NC is None:
        _NC = _build()
    return _NC


def _tok_idx(c):
    """Global token indices owned by core c (batch-interleaved)."""
    return np.r_[TB * c:TB * (c + 1), S * 1 * B // 2 + TB * c:2048 + TB * (c + 1)]


def prepare_in_maps(inputs):
    bf = ml_dtypes.bfloat16
    f8 = ml_dtypes.float8_e4m3
    inp = {k: np.ascontiguousarray(np.asarray(v, dtype=np.float32)) for k, v in inputs.items()}
    x_flat = inp["x"].reshape(BS, D)
    wcols = {
        "wfq": inp["fqk_w_Q"].reshape(BS, N), "wfk": inp["fqk_w_K"].reshape(BS, N),
        "wfv": inp["fv_w"].reshape(BS, N), "wkf": inp["feature_know_w"].reshape(BS, N),
    }
    wrows = {
        "wrqT": inp["rqk_w_Q"].reshape(BS, N), "wrkT": inp["rqk_w_K"].reshape(BS, N),
        "wrvT": inp["rv_w"].reshape(BS, N), "wkrT": inp["restore_know_w"].reshape(BS, N),
    }
    g1 = inp["ln1_g"][None, :, None]
    g2 = inp["ln2_g"][None, :, None]
    assert np.abs(inp["ln1_b"]).max() == 0 and np.abs(inp["ln2_b"]).max() == 0, \
        "nonzero LN bias not supported by this build"
    pools = {
        "fqk_p": (inp["f_qk"] * g1).astype(f8), "fv_p": (inp["f_v"] * g1).astype(f8),
        "rqk_p": inp["r_qk"].astype(f8), "rv_p": inp["r_v"].astype(f8),
        "fkn_p": (inp["f_know"] * g2).astype(f8), "rkn_p": inp["r_know"].astype(f8),
    }
    wo_p = np.ascontiguousarray(inp["W_O"].T).astype(f8)
    masku = np.ascontiguousarray(np.tril(np.ones((P, P), np.float32)).T).astype(f8)

    in_maps = []
    for c in range(NC):
        idx = np.r_[TB * c:TB * (c + 1), S + TB * c:S + TB * (c + 1)]
        m = {
            "x_sh": np.ascontiguousarray(x_flat[idx]),
            "wo_p": wo_p, "masku": masku,
        }
        m.update(pools)
        for k, v in wcols.items():
            m[k] = np.ascontiguousarray(v[idx])
        for k, v in wrows.items():
            m[k] = np.ascontiguousarray(v[idx].T).astype(bf)
        in_maps.append(m)
    return in_maps


def kernel(**inputs):
    nc = _get_nc()
    in_maps = prepare_in_maps(inputs)
    res = run_bass_kernel_spmd(nc, in_maps, list(range(NC))).results
    out = np.zeros((BS, D), np.float32)
    for c in range(NC):
        out[TB * c:TB * (c + 1)] = res[c]["out_sh"][0:TB]
        out[S + TB * c:S + TB * (c + 1)] = res[c]["out_sh"][TB:T]
    return out.reshape(B, S, D)


# revision 28
# speedup vs baseline: 1.0290x; 1.0205x over previous
"""Trainium2 Bass kernel for nn_DAWNBlock (8-core SPMD), v5.

Decomposition (validated in numpy: proto_check.py, quant_check.py):
  - Token-sharded with batch-interleaved ownership: core c owns global tokens
    [256c, 256c+256) of batch 0 AND [256c, 256c+256) of batch 1 (T=512).
    LN1, features, restores (Q/K/V), LN2 + knowledge run token-local.
  - Head-sharded attention: core c owns d-cols [128c, 128c+128) (heads
    {2c, 2c+1}). One A2A reshards Q^T+K^T (1MB fp8), one reshards V (0.5MB).
    Attention output returns via TWO 0.25MB A2As (one per batch) so batch 0's
    resharding and its W_O hide under batch 1's attention.
  - fp8e4m3 + DoubleRow (K=256 pairs, 2x PE rate) for features, restores,
    PV, W_O, knowledge. Scores fp8 K=64 packed as concurrent row-group pairs.
    All-fp8 quantization study: 4e-3 rel err vs the 2e-2 gate.
  - LN gains folded into the f-pools host-side (bias asserted zero); LN is
    stats + one ACT apply. Routing weights: feature PSUM banks combined via
    DVE scalar_tensor_tensor; restores use A[n] = hT * wbc[n] (PE-broadcast
    rows), h transposed once.
  - Causal softmax without max-subtraction; denominator via ones-column in V;
    o_ps evacuated to SBUF immediately to release PSUM banks.
"""
import sys

sys.path.insert(0, "/opt/trn_rl_repo")

import os
import numpy as np
import ml_dtypes
import concourse.bass as bass
import concourse.mybir as mybir
import concourse.tile as tile
from concourse import bacc
from concourse.bass_utils import run_bass_kernel_spmd
from concourse.masks import make_identity

B, S, D, H, R, N, KR = 2, 2048, 1024, 16, 256, 8, 128
DH = D // H           # 64
BS = B * S            # 4096
NC = 8
T = BS // NC          # 512 tokens per core (256 per batch)
TB = T // B           # 256 tokens per (core, batch)
P = 128
TT = T // P           # 4 token tiles per core
DC = D // P           # 8 d chunks
DCP = DC // 2         # 4 d chunk-pairs (DoubleRow)
RC = R // P           # 2 r chunks
EPS = 1e-5

STAGES = int(os.environ.get("BASS_STAGES", "5"))
F32 = mybir.dt.float32
BF = mybir.dt.bfloat16
F8 = mybir.dt.float8e4
DR = mybir.MatmulPerfMode.DoubleRow
AF = mybir.ActivationFunctionType
OP = mybir.AluOpType


def _layernorm(nc, cpool, x_sb, nx_sb, eps_tile, tag):
    """nx = (x - mean(x)) * rsqrt(var + eps) for one [128, D] tile.
    (LN gain folded into downstream pools host-side; bias asserted zero.)"""
    s = cpool.tile([P, 1], F32, tag="ln_s", name=f"{tag}_s")
    nm = cpool.tile([P, 1], F32, tag="ln_nm", name=f"{tag}_nm")
    sq = cpool.tile([P, D], F32, tag="ln_sq", name=f"{tag}_sq")
    ssq = cpool.tile([P, 1], F32, tag="ln_ssq", name=f"{tag}_ssq")
    rs = cpool.tile([P, 1], F32, tag="ln_rs", name=f"{tag}_rs")
    nmrs = cpool.tile([P, 1], F32, tag="ln_nmrs", name=f"{tag}_nmrs")
    nc.vector.reduce_sum(s[:], x_sb[:], axis=mybir.AxisListType.X)
    nc.vector.tensor_scalar_mul(nm[:], s[:], -1.0 / D)
    nc.scalar.activation(sq[:], x_sb[:], AF.Square, bias=nm[:], accum_out=ssq[:])
    nc.scalar.activation(rs[:], ssq[:], AF.Abs_reciprocal_sqrt,
                         bias=eps_tile[:], scale=1.0 / D)
    nc.vector.tensor_mul(nmrs[:], nm[:], rs[:])
    nc.scalar.activation(nx_sb[:], x_sb[:], AF.Identity, bias=nmrs[:], scale=rs[:])


def _build():
    nc = bacc.Bacc("TRN2", target_bir_lowering=False, debug=False, num_devices=NC)

    def di(name, shape, dt=F8):
        return nc.dram_tensor(name, shape, dt, kind="ExternalInput").ap()

    x_in = di("x_sh", [T, D], F32)
    wcol_in = {k: di(k, [T, N], F32) for k in ["wfq", "wfk", "wfv", "wkf"]}
    wrow_in = {k: di(k, [N, T], BF) for k in ["wrqT", "wrkT", "wrvT", "wkrT"]}
    fqk_in = di("fqk_p", [N, D, R])
    fv_in = di("fv_p", [N, D, R])
    rqk_in = di("rqk_p", [N, R, D])
    rv_in = di("rv_p", [N, R, D])
    fkn_in = di("fkn_p", [N, D, KR])
    rkn_in = di("rkn_p", [N, KR, D])
    wo_in = di("wo_p", [D, D])       # = W_O.T
    masku_in = di("masku", [P, P])
    out_ap = nc.dram_tensor("out_sh", [T, D], F32, kind="ExternalOutput").ap()

    with tile.TileContext(nc) as tc:
        from contextlib import ExitStack
        with ExitStack() as ctx:
            const = ctx.enter_context(tc.tile_pool(name="const", bufs=1))
            cpool = ctx.enter_context(tc.tile_pool(name="scratch", bufs=2))
            dram = ctx.enter_context(tc.tile_pool(name="dram", bufs=1, space="DRAM"))

            # ---------- pools (stack order: long-lived first) ----------
            xpool = ctx.enter_context(tc.tile_pool(name="xpool", bufs=1))
            x_t = [xpool.tile([P, D], F32, tag=f"x{tt}", name=f"x{tt}") for tt in range(TT)]

            cprep = ctx.enter_context(tc.tile_pool(name="cprep", bufs=1))
            wo_sb = cprep.tile([P, DCP, 2, D], F8, tag="wo_sb", name="wo_sb")
            fkn_sb = cprep.tile([P, DCP, 2, N, KR], F8, tag="fkn_sb", name="fkn_sb")
            rkn_sb = cprep.tile([P, N, D], F8, tag="rkn_sb", name="rkn_sb")
            wbc_kr = []

            hpool = ctx.enter_context(tc.tile_pool(name="hpool", bufs=1))
            h_q = [hpool.tile([P, R], F32, tag=f"hq{tt}", name=f"hq{tt}") for tt in range(TT)]
            h_k = [hpool.tile([P, R], F32, tag=f"hk{tt}", name=f"hk{tt}") for tt in range(TT)]
            h_v = [hpool.tile([P, R], F32, tag=f"hv{tt}", name=f"hv{tt}") for tt in range(TT)]
            hbf_q = [hpool.tile([P, R], BF, tag=f"hbq{tt}", name=f"hbq{tt}") for tt in range(TT)]
            hbf_k = [hpool.tile([P, R], BF, tag=f"hbk{tt}", name=f"hbk{tt}") for tt in range(TT)]
            hbf_v = [hpool.tile([P, R], BF, tag=f"hbv{tt}", name=f"hbv{tt}") for tt in range(TT)]

            from contextlib import ExitStack as _ES
            rp_ctx = _ES()
            rp = rp_ctx.enter_context(tc.tile_pool(name="rp", bufs=1))
            rqk_sb = rp.tile([P, N, RC, D], F8, tag="rqk_sb", name="rqk_sb")
            rv_sb = rp.tile([P, N, RC, D], F8, tag="rv_sb", name="rv_sb")

            fw_ctx = _ES()
            fwpool = fw_ctx.enter_context(tc.tile_pool(name="fwpool", bufs=1))
            fqk_sb = [fwpool.tile([P, 2, N, R], F8, tag=f"fqk{dcp}", name=f"fqk{dcp}")
                      for dcp in range(DCP)]
            fv_sb = [fwpool.tile([P, 2, N, R], F8, tag=f"fv{dcp}", name=f"fv{dcp}")
                     for dcp in range(DCP)]

            # ---------- big prefetches in priority order ----------
            for tt in range(TT):
                nc.sync.dma_start(x_t[tt][:], x_in[P * tt:P * (tt + 1), :])
            for dcp in range(DCP):
                for i in range(2):
                    dc = 2 * dcp + i
                    nc.sync.dma_start(
                        fqk_sb[dcp][:, i],
                        fqk_in[:, P * dc:P * (dc + 1), :].rearrange("n p r -> p n r"))
                    nc.sync.dma_start(
                        fv_sb[dcp][:, i],
                        fv_in[:, P * dc:P * (dc + 1), :].rearrange("n p r -> p n r"))
            for n in range(N):
                nc.sync.dma_start(
                    rqk_sb[:, n], rqk_in[n].rearrange("(rc p) d -> p rc d", p=P))
                nc.sync.dma_start(
                    rv_sb[:, n], rv_in[n].rearrange("(rc p) d -> p rc d", p=P))
            # tiny warm-up collective: pays the first-collective ncfw latency
            dumm_b = dram.tile([NC * 8, 8], F32, tag="dumm_b", name="dumm_b")
            dumm_o = dram.tile([NC * 8, 8], F32, tag="dumm_o", name="dumm_o")
            nc.gpsimd.collective_compute(
                "AllToAll", OP.bypass, replica_groups=[list(range(NC))],
                ins=[dumm_b.opt()], outs=[dumm_o.opt()])

            ident_f = const.tile([P, P], F32, tag="ident_f", name="ident_f")
            make_identity(nc, ident_f)
            identb = const.tile([P, P], BF, tag="identb", name="identb")
            nc.vector.tensor_copy(identb[:], ident_f[:])
            ones_bf = const.tile([1, P], BF, tag="ones_bf", name="ones_bf")
            nc.vector.memset(ones_bf[:], 1.0)
            masku = const.tile([P, P], F8, tag="masku", name="masku")
            nc.sync.dma_start(masku[:], masku_in[:])
            eps_t = const.tile([P, 1], F32, tag="eps", name="eps")
            nc.vector.memset(eps_t[:], EPS)

            wsb = {}
            for k in wcol_in:
                wt = const.tile([P, TT, N], F32, tag=f"w_{k}", name=f"w_{k}")
                nc.sync.dma_start(wt[:], wcol_in[k].rearrange("(tt p) n -> p tt n", p=P))
                wsb[k] = wt

            # ---------- DRAM bounce buffers for collectives ----------
            qk_b = dram.tile([NC * 2 * P, T], F8, tag="qk_b", name="qk_b")
            qk_o = dram.tile([NC * 2 * P, T], F8, tag="qk_o", name="qk_o")
            v_b = dram.tile([BS, P], F8, tag="v_b", name="v_b")
            v_o = dram.tile([BS, P], F8, tag="v_o", name="v_o")
            ab_bs = [dram.tile([NC * P, TB], F8, tag=f"ab_b{b}", name=f"ab_b{b}")
                     for b in range(B)]
            ab_os = [dram.tile([NC * P, TB], F8, tag=f"ab_o{b}", name=f"ab_o{b}")
                     for b in range(B)]

            # ================= stage A: LN1 + transpose + features ========
            with tc.tile_pool(name="stagea", bufs=1) as stagea:
                nxT = {}
                with tc.tile_pool(name="ps_tr", bufs=2, space="PSUM") as ps_tr:
                    for tt in range(TT):
                        nx_t = stagea.tile([P, D], BF, tag="nxa", name=f"nxa{tt}", bufs=2)
                        _layernorm(nc, cpool, x_t[tt], nx_t, eps_t, f"ln1_{tt}")
                        for dcp in range(DCP):
                            pst = ps_tr.tile([P, 2, P], BF, tag="tp", name="tp")
                            for i in range(2):
                                dc = 2 * dcp + i
                                nc.tensor.transpose(pst[:, i], nx_t[:, P * dc:P * (dc + 1)],
                                                    identb[:])
                            t8 = stagea.tile([P, 2, P], F8, tag=f"nxT{tt}_{dcp}",
                                             name=f"nxT{tt}_{dcp}")
                            nc.vector.tensor_copy(t8[:], pst[:])
                            nxT[(tt, dcp)] = t8

                with tc.tile_pool(name="ps_feat", bufs=8, space="PSUM") as ps_feat:
                    for tt in range(TT):
                        psf = [ps_feat.tile([P, 512], F32, tag="feat", name=f"feat{tt}_{g}")
                               for g in range(8)]
                        for dcp in range(DCP):
                            lhs = nxT[(tt, dcp)][:]
                            for g in range(8):
                                src = (fqk_sb if g < 4 else fv_sb)[dcp]
                                gg = g % 4
                                nc.tensor.matmul(
                                    psf[g][:], lhs, src[:, :, 2 * gg:2 * (gg + 1), :],
                                    start=(dcp == 0), stop=(dcp == DCP - 1), perf_mode=DR)
                        for m in range(N):
                            bank, half = m // 2, m % 2
                            pq = psf[bank][:, R * half:R * (half + 1)]
                            pv = psf[4 + bank][:, R * half:R * (half + 1)]
                            for htiles, hbfs, wk, ps_slice in (
                                    (h_q, hbf_q, "wfq", pq), (h_k, hbf_k, "wfk", pq),
                                    (h_v, hbf_v, "wfv", pv)):
                                wcol = wsb[wk][:, tt, m:m + 1]
                                if m == 0:
                                    nc.vector.tensor_scalar_mul(htiles[tt][:], ps_slice, wcol)
                                elif m == N - 1:
                                    nc.vector.scalar_tensor_tensor(
                                        hbfs[tt][:], ps_slice, wcol, htiles[tt][:],
                                        op0=OP.mult, op1=OP.add)
                                else:
                                    nc.vector.scalar_tensor_tensor(
                                        htiles[tt][:], ps_slice, wcol, htiles[tt][:],
                                        op0=OP.mult, op1=OP.add)

            fw_ctx.close()

            if STAGES == 1:
                for tt in range(TT):
                    hq_ev = cpool.tile([P, R], F32, tag="hq_ev", name=f"hq_ev{tt}")
                    nc.vector.tensor_copy(hq_ev[:], h_q[tt][:])
                    nc.sync.dma_start(out_ap[P * tt:P * (tt + 1), 0:R], hq_ev[:])

            # ================= stage B: restores + A2A =====================
            if STAGES >= 2:
                with tc.tile_pool(name="bpool", bufs=1) as bpool, \
                     tc.tile_pool(name="ap_pool", bufs=2) as ap_pool, \
                     tc.tile_pool(name="ps_bt", bufs=2, space="PSUM") as ps_bt, \
                     tc.tile_pool(name="ps_bc", bufs=2, space="PSUM") as ps_bc, \
                     tc.tile_pool(name="ps_r", bufs=4, space="PSUM") as ps_r, \
                     tc.tile_pool(name="ev_b", bufs=3) as ev_b:
                    wbc = {}
                    for k in ["wrqT", "wrkT", "wrvT"]:
                        tiles = []
                        for n in range(N):
                            rowt = bpool.tile([1, T], BF, tag="wrow", name=f"{k}row{n}", bufs=2)
                            nc.sync.dma_start(rowt[:], wrow_in[k][n:n + 1, :])
                            psb = ps_bc.tile([P, T], F32, tag="wbc_ps", name="wbc_ps")
                            nc.tensor.matmul(psb[:], ones_bf[:], rowt[:])
                            wt = bpool.tile([P, T], BF, tag=f"wbc_{k}", name=f"wbc_{k}{n}")
                            nc.scalar.activation(wt[:], psb[:], AF.Copy)
                            tiles.append(wt)
                        wbc[k] = tiles
                    hT = {}
                    for key, hbfs in (("q", hbf_q), ("k", hbf_k), ("v", hbf_v)):
                        ht = bpool.tile([P, RC, T], BF, tag=f"hT{key}", name=f"hT{key}")
                        for tt in range(TT):
                            pst = ps_bt.tile([P, 2, P], BF, tag="tpb", name="tpb")
                            for rc in range(RC):
                                nc.tensor.transpose(pst[:, rc],
                                                    hbfs[tt][:, P * rc:P * (rc + 1)],
                                                    identb[:])
                            nc.vector.tensor_copy(ht[:, :, P * tt:P * (tt + 1)], pst[:])
                        hT[key] = ht

                    def build_A(hkey, wkey):
                        A = [ap_pool.tile([P, RC, T], F8, tag=f"A{n}", name=f"A_{wkey}{n}")
                             for n in range(N)]
                        for n in range(N):
                            for rc in range(RC):
                                nc.vector.tensor_mul(
                                    A[n][:, rc, :], hT[hkey][:, rc, :], wbc[wkey][n][:])
                        return A

                    def qk_restore(A, row_off):
                        for dm in range(DC):
                            ps = ps_r.tile([P, T], F32, tag="r_ps", name="r_ps")
                            for n in range(N):
                                nc.tensor.matmul(
                                    ps[:], rqk_sb[:, n, :, P * dm:P * (dm + 1)], A[n][:],
                                    start=(n == 0), stop=(n == N - 1), perf_mode=DR)
                            ev = ev_b.tile([P, T], F8, tag="ev_qk", name="ev_qk")
                            nc.scalar.activation(ev[:], ps[:], AF.Copy)
                            nc.sync.dma_start(
                                qk_b[2 * P * dm + row_off: 2 * P * dm + row_off + P, :],
                                ev[:])

                    Aq = build_A("q", "wrqT")
                    Ak = build_A("k", "wrkT")
                    Av = build_A("v", "wrvT")
                    qk_restore(Aq, 0)
                    qk_restore(Ak, P)
                    cc_qk = nc.gpsimd.collective_compute(
                        "AllToAll", OP.bypass, replica_groups=[list(range(NC))],
                        ins=[qk_b.opt()], outs=[qk_o.opt()])
                    first_v_mm = [None]
                    for tt in range(TT):
                        for jf in range(2):
                            ps = ps_r.tile([P, 512], F32, tag="r_ps", name="v_ps")
                            for n in range(N):
                                mm = nc.tensor.matmul(
                                    ps[:], Av[n][:, :, P * tt:P * (tt + 1)],
                                    rv_sb[:, n, :, 512 * jf:512 * (jf + 1)],
                                    start=(n == 0), stop=(n == N - 1), perf_mode=DR)
                                if first_v_mm[0] is None:
                                    first_v_mm[0] = mm
                            ev = ev_b.tile([P, 512], F8, tag="ev_v", name="ev_v")
                            nc.vector.tensor_copy(ev[:], ps[:])
                            for db in range(4):
                                d = 4 * jf + db
                                nc.sync.dma_start(
                                    v_b[T * d + P * tt: T * d + P * (tt + 1), :],
                                    ev[:, P * db:P * (db + 1)])
                    nc.gpsimd.collective_compute(
                        "AllToAll", OP.bypass, replica_groups=[list(range(NC))],
                        ins=[v_b.opt()], outs=[v_o.opt()])
                    # hold V-restore matmuls until the qk collective trigger has
                    # fired, so they execute inside its completion window
                    from concourse.tile import add_dep_helper as _adh
                    try:
                        _adh(cc_qk.ins if hasattr(cc_qk, "ins") else cc_qk,
                             first_v_mm[0].ins if hasattr(first_v_mm[0], "ins")
                             else first_v_mm[0],
                             sync=True, reason="V restore covers qk A2A window")
                    except Exception:
                        pass
                    for n in range(N):
                        rowt = bpool.tile([1, T], BF, tag="wrow", name=f"krrow{n}", bufs=2)
                        nc.sync.dma_start(rowt[:], wrow_in["wkrT"][n:n + 1, :])
                        psb = ps_bc.tile([P, T], F32, tag="wbc_ps", name="wbckr_ps")
                        nc.tensor.matmul(psb[:], ones_bf[:], rowt[:])
                        wt = cprep.tile([P, T], BF, tag=f"wbc_kr{n}", name=f"wbc_kr{n}")
                        nc.scalar.activation(wt[:], psb[:], AF.Copy)
                        wbc_kr.append(wt)

            rp_ctx.close()

            if STAGES == 2:
                for tt in range(TT):
                    qo_ev = cpool.tile([P, T], F8, tag="qo_ev", name=f"qo_ev{tt}")
                    nc.sync.dma_start(qo_ev[:], qk_o[2 * P * tt:2 * P * tt + P, :])
                    qo_f = cpool.tile([P, T], F32, tag="qo_f", name=f"qo_f{tt}")
                    nc.vector.tensor_copy(qo_f[:], qo_ev[:])
                    nc.sync.dma_start(out_ap[P * tt:P * (tt + 1), 0:T], qo_f[:])

            # ========== attention (head-sharded, packed) + W_O + stage C ===
            if STAGES >= 3:
                cpers = ctx.enter_context(tc.tile_pool(name="cpers", bufs=1))
                cscr = ctx.enter_context(tc.tile_pool(name="cscr", bufs=2))
                ps_acc = ctx.enter_context(
                    tc.tile_pool(name="ps_acc", bufs=2, space="PSUM"))
                aTs = [cpers.tile([P, DCP, 2, TB], F8, tag=f"aT{b}", name=f"aT{b}")
                       for b in range(B)]
                x2 = [cpers.tile([P, D], F32, tag=f"x2_{tt}", name=f"x2_{tt}")
                      for tt in range(TT)]

                def wo_block(tt):
                    """W_O for token tile tt (tokens of batch tt//2)."""
                    hb, off = tt // 2, P * (tt % 2)
                    for jf in range(2):
                        ps = ps_acc.tile([P, 512], F32, tag="acc", name=f"wo_ps{tt}")
                        for dcp in range(DCP):
                            nc.tensor.matmul(
                                ps[:], aTs[hb][:, dcp, :, off:off + P],
                                wo_sb[:, dcp, :, 512 * jf:512 * (jf + 1)],
                                start=(dcp == 0), stop=(dcp == DCP - 1), perf_mode=DR)
                        nc.vector.tensor_add(
                            x2[tt][:, 512 * jf:512 * (jf + 1)],
                            x_t[tt][:, 512 * jf:512 * (jf + 1)], ps[:])

                with tc.tile_pool(name="qkv_bh", bufs=1) as qkv_bh, \
                     tc.tile_pool(name="pt_pool", bufs=20) as pt_pool, \
                     tc.tile_pool(name="osb_pool", bufs=4) as osb_pool, \
                     tc.tile_pool(name="ps_st", bufs=4, space="PSUM") as ps_st, \
                     tc.tile_pool(name="ps_o", bufs=2, space="PSUM") as ps_o:
                    # all q/k loads first (vp loads wait on the V collective and
                    # would otherwise block batch-1 q/k behind them in the queue);
                    # fine-grained tiles so early score matmuls start per-chunk
                    qtg, ktc, vpss = {}, {}, []
                    for b in range(B):
                        for qg in range(4):
                            t = qkv_bh.tile([P, 512], F8, tag=f"qtg{b}_{qg}",
                                            name=f"qtg{b}_{qg}")
                            for half in range(2):
                                c = 2 * qg + half
                                nc.sync.dma_start(
                                    t[:, TB * half:TB * (half + 1)],
                                    qk_o[2 * P * c:2 * P * c + P, TB * b:TB * (b + 1)])
                            qtg[(b, qg)] = t
                        for c in range(NC):
                            kt_t = qkv_bh.tile([P, TB], F8, tag=f"ktc{b}_{c}",
                                               name=f"ktc{b}_{c}")
                            nc.sync.dma_start(
                                kt_t[:], qk_o[2 * P * c + P:2 * P * (c + 1),
                                              TB * b:TB * (b + 1)])
                            ktc[(b, c)] = kt_t
                    for b in range(B):
                        vps = []
                        for h2 in range(2):
                            vp = qkv_bh.tile([P, S // P // 2, 2, 80], F8,
                                             tag=f"vp{h2}_{b}", name=f"vp{h2}_{b}")
                            src = v_o.rearrange("(c hb i p) f -> hb p c i f",
                                                hb=2, i=2, p=P)[b]
                            for i in range(2):
                                nc.sync.dma_start(
                                    vp[:, :, i, 0:DH],
                                    src[:, :, i, DH * h2:DH * (h2 + 1)])
                            nc.vector.memset(vp[:, :, :, DH:DH + 1], 1.0)
                            vps.append(vp)
                        vpss.append(vps)
                    # stage-C weights: needed only after attention; emitted after
                    # the attention loads so they don't block those DMA queues
                    for dcp in range(DCP):
                        for i in range(2):
                            dc = 2 * dcp + i
                            nc.sync.dma_start(wo_sb[:, dcp, i],
                                              wo_in[P * dc:P * (dc + 1), :])
                            nc.sync.dma_start(
                                fkn_sb[:, dcp, i],
                                fkn_in[:, P * dc:P * (dc + 1), :].rearrange("n p f -> p n f"))
                    for n in range(N):
                        nc.sync.dma_start(rkn_sb[:, n], rkn_in[n])

                    for b in range(B):
                        vps = vpss[b]
                        for qg in range(4):
                            o_ps = [ps_o.tile([DH + 1, 512], F32, tag="o_ps",
                                              name=f"o{b}_{qg}_{h2}") for h2 in range(2)]
                            nkt = 4 * qg + 4
                            for u in range(nkt // 2):
                                pt2 = [pt_pool.tile([P, 2, 512], F8, tag="pt",
                                                    name=f"pt{b}_{qg}_{u}_{h2}")
                                       for h2 in range(2)]
                                j0 = 2 * u - 4 * qg   # j of the pair's first chunk
                                vs = max(0, P * j0)   # valid query start for the pair
                                for i in range(2):
                                    kt = 2 * u + i
                                    j = kt - 4 * qg
                                    qs = max(0, P * j)  # this chunk's valid query start
                                    for h2 in range(2):
                                        st = ps_st.tile([P, 512], F32, tag="st", name="st")
                                        koff = P * (kt % 2)
                                        nc.tensor.matmul(
                                            st[:, qs:],
                                            ktc[(b, kt // 2)][DH * h2:DH * (h2 + 1),
                                                              koff:koff + P],
                                            qtg[(b, qg)][DH * h2:DH * (h2 + 1), qs:])
                                        pt = pt2[h2][:, i, :]
                                        if j < 0:
                                            nc.scalar.activation(pt, st[:], AF.Exp,
                                                                 scale=0.125)
                                        else:
                                            if qs > vs:
                                                # zero only the slice PV will read
                                                nc.vector.memset(pt[:, vs:qs], 0.0)
                                            nc.scalar.activation(
                                                pt[:, qs:], st[:, qs:],
                                                AF.Exp, scale=0.125)
                                            nc.vector.tensor_mul(
                                                pt[:, P * j:P * (j + 1)],
                                                pt[:, P * j:P * (j + 1)], masku[:])
                                for h2 in range(2):
                                    nc.tensor.matmul(
                                        o_ps[h2][:, vs:], vps[h2][:, u, :, 0:DH + 1],
                                        pt2[h2][:, :, vs:],
                                        start=(u == 0), stop=(u == nkt // 2 - 1),
                                        perf_mode=DR, skip_group_check=True)
                            for h2 in range(2):
                                # evacuate PSUM immediately; normalize from SBUF
                                den = cpool.tile([1, 512], F32, tag="den", name="den")
                                nc.vector.tensor_copy(den[:], o_ps[h2][DH:DH + 1, :])
                                o_sb = osb_pool.tile([DH, 512], BF, tag="o_sb",
                                                     name=f"osb{b}_{qg}_{h2}")
                                nc.vector.tensor_copy(o_sb[:], o_ps[h2][0:DH, :])
                                rec_f = cpool.tile([1, 512], F32, tag="rec_f", name="rec_f")
                                nc.vector.reciprocal_approx_fast(rec_f[:], den[:])
                                rec = cpool.tile([1, 512], BF, tag="rec", name="rec")
                                nc.vector.tensor_copy(rec[:], rec_f[:])
                                bc = ps_st.tile([DH, 512], F32, tag="st", name="bc")
                                nc.tensor.matmul(bc[:], ones_bf[:, 0:DH], rec[:])
                                bc_sb = cpool.tile([DH, 512], BF, tag="bc_sb", name="bc_sb")
                                nc.vector.tensor_copy(bc_sb[:], bc[:])
                                nrm = cpool.tile([DH, 512], F8, tag="nrm", name="nrm")
                                nc.vector.tensor_mul(nrm[:], o_sb[:], bc_sb[:])
                                for half in range(2):
                                    cblk = 2 * qg + half
                                    nc.sync.dma_start(
                                        ab_bs[b][P * cblk + DH * h2:
                                                 P * cblk + DH * (h2 + 1), :],
                                        nrm[:, TB * half:TB * (half + 1)])
                        nc.gpsimd.collective_compute(
                            "AllToAll", OP.bypass, replica_groups=[list(range(NC))],
                            ins=[ab_bs[b].opt()], outs=[ab_os[b].opt()])
                        for dcp in range(DCP):
                            for i in range(2):
                                dc = 2 * dcp + i
                                nc.sync.dma_start(aTs[b][:, dcp, i],
                                                  ab_os[b][P * dc:P * (dc + 1), :])
                        if b == 1:
                            # W_O for batch 0 tokens: fills the PE while the
                            # batch-1 A2A completes
                            wo_block(0)
                            wo_block(1)

                if STAGES == 3:
                    for tt in range(TT):
                        at_f = cpool.tile([P, D], F32, tag="at_f", name=f"at_f{tt}")
                        nc.vector.tensor_copy(at_f[:], x2[tt % 2][:])
                        nc.sync.dma_start(out_ap[P * tt:P * (tt + 1), :], at_f[:])

                # ---------------- stage C ----------------
                if STAGES >= 4:
                    with tc.tile_pool(name="ps_ct", bufs=2, space="PSUM") as ps_ct, \
                         tc.tile_pool(name="ps_kf", bufs=2, space="PSUM") as ps_kf, \
                         tc.tile_pool(name="ps_kr", bufs=2, space="PSUM") as ps_kr:
                        wo_block(2)
                        wo_block(3)
                        nx2T = {}
                        hknT = cpers.tile([P, T], BF, tag="hknT", name="hknT")
                        for tt in range(TT):
                            if STAGES >= 5:
                                nx2 = cscr.tile([P, D], BF, tag="nx2", name=f"nx2_{tt}")
                                _layernorm(nc, cpool, x2[tt], nx2, eps_t, f"ln2_{tt}")
                                for dcp in range(DCP):
                                    pst = ps_ct.tile([P, 2, P], BF, tag="tpc", name="tpc")
                                    for i in range(2):
                                        dc = 2 * dcp + i
                                        nc.tensor.transpose(
                                            pst[:, i], nx2[:, P * dc:P * (dc + 1)], identb[:])
                                    t8 = cscr.tile([P, 2, P], F8, tag=f"nx2T{tt}_{dcp}",
                                                   name=f"nx2T{tt}_{dcp}", bufs=1)
                                    nc.vector.tensor_copy(t8[:], pst[:])
                                    nx2T[(tt, dcp)] = t8

                        if STAGES == 4:
                            for tt in range(TT):
                                ao_ev = cpool.tile([P, D], F32, tag="ao_ev", name=f"ao_ev{tt}")
                                nc.vector.tensor_copy(ao_ev[:], x2[tt][:])
                                nc.sync.dma_start(out_ap[P * tt:P * (tt + 1), :], ao_ev[:])

                        if STAGES >= 5:
                            for tt in range(TT):
                                psk = [ps_kf.tile([P, 512], F32, tag="kf", name=f"kf{tt}_{g}")
                                       for g in range(2)]
                                for dcp in range(DCP):
                                    lhs = nx2T[(tt, dcp)][:]
                                    for g in range(2):
                                        nc.tensor.matmul(
                                            psk[g][:], lhs,
                                            fkn_sb[:, dcp, :, 4 * g:4 * (g + 1), :],
                                            start=(dcp == 0), stop=(dcp == DCP - 1),
                                            perf_mode=DR)
                                hkn = cscr.tile([P, KR], F32, tag="hkn", name=f"hkn{tt}")
                                for m in range(N):
                                    pslice = psk[m // 4][:, KR * (m % 4):KR * (m % 4 + 1)]
                                    wcol = wsb["wkf"][:, tt, m:m + 1]
                                    if m == 0:
                                        nc.vector.tensor_scalar_mul(hkn[:], pslice, wcol)
                                    else:
                                        nc.vector.scalar_tensor_tensor(
                                            hkn[:], pslice, wcol, hkn[:],
                                            op0=OP.mult, op1=OP.add)
                                hknb = cscr.tile([P, KR], BF, tag="hknb", name=f"hknb{tt}")
                                nc.vector.tensor_copy(hknb[:], hkn[:])
                                pst = ps_ct.tile([P, P], BF, tag="tpc", name="tpc_kn")
                                nc.tensor.transpose(pst[:], hknb[:], identb[:])
                                nc.vector.tensor_copy(hknT[:, P * tt:P * (tt + 1)], pst[:])

                            Akn = cpers.tile([P, N, T], F8, tag="Akn", name="Akn")
                            for n in range(N):
                                nc.vector.tensor_mul(Akn[:, n, :], hknT[:], wbc_kr[n][:])
                            for tt in range(TT):
                                for jf in range(2):
                                    ps = ps_kr.tile([P, 512], F32, tag="kr_ps", name="kn_ps")
                                    for u in range(N // 2):
                                        nc.tensor.matmul(
                                            ps[:],
                                            Akn[:, 2 * u:2 * (u + 1), P * tt:P * (tt + 1)],
                                            rkn_sb[:, 2 * u:2 * (u + 1),
                                                   512 * jf:512 * (jf + 1)],
                                            start=(u == 0), stop=(u == N // 2 - 1),
                                            perf_mode=DR)
                                    out_sb = cscr.tile([P, 512], F32, tag="out_sb",
                                                       name="out_sb")
                                    nc.vector.tensor_add(
                                        out_sb[:], x2[tt][:, 512 * jf:512 * (jf + 1)], ps[:])
                                    nc.sync.dma_start(
                                        out_ap[P * tt:P * (tt + 1),
                                               512 * jf:512 * (jf + 1)],
                                        out_sb[:])

    nc.compile()
    return nc


_NC = None


def _get_nc():
    global _NC
    if _NC is None:
        _NC = _build()
    return _NC


def _tok_idx(c):
    """Global token indices owned by core c (batch-interleaved)."""
    return np.r_[TB * c:TB * (c + 1), S * 1 * B // 2 + TB * c:2048 + TB * (c + 1)]


def prepare_in_maps(inputs):
    bf = ml_dtypes.bfloat16
    f8 = ml_dtypes.float8_e4m3
    inp = {k: np.ascontiguousarray(np.asarray(v, dtype=np.float32)) for k, v in inputs.items()}
    x_flat = inp["x"].reshape(BS, D)
    wcols = {
        "wfq": inp["fqk_w_Q"].reshape(BS, N), "wfk": inp["fqk_w_K"].reshape(BS, N),
        "wfv": inp["fv_w"].reshape(BS, N), "wkf": inp["feature_know_w"].reshape(BS, N),
    }
    wrows = {
        "wrqT": inp["rqk_w_Q"].reshape(BS, N), "wrkT": inp["rqk_w_K"].reshape(BS, N),
        "wrvT": inp["rv_w"].reshape(BS, N), "wkrT": inp["restore_know_w"].reshape(BS, N),
    }
    g1 = inp["ln1_g"][None, :, None]
    g2 = inp["ln2_g"][None, :, None]
    assert np.abs(inp["ln1_b"]).max() == 0 and np.abs(inp["ln2_b"]).max() == 0, \
        "nonzero LN bias not supported by this build"
    pools = {
        "fqk_p": (inp["f_qk"] * g1).astype(f8), "fv_p": (inp["f_v"] * g1).astype(f8),
        "rqk_p": inp["r_qk"].astype(f8), "rv_p": inp["r_v"].astype(f8),
        "fkn_p": (inp["f_know"] * g2).astype(f8), "rkn_p": inp["r_know"].astype(f8),
    }
    wo_p = np.ascontiguousarray(inp["W_O"].T).astype(f8)
    masku = np.ascontiguousarray(np.tril(np.ones((P, P), np.float32)).T).astype(f8)

    in_maps = []
    for c in range(NC):
        idx = np.r_[TB * c:TB * (c + 1), S + TB * c:S + TB * (c + 1)]
        m = {
            "x_sh": np.ascontiguousarray(x_flat[idx]),
            "wo_p": wo_p, "masku": masku,
        }
        m.update(pools)
        for k, v in wcols.items():
            m[k] = np.ascontiguousarray(v[idx])
        for k, v in wrows.items():
            m[k] = np.ascontiguousarray(v[idx].T).astype(bf)
        in_maps.append(m)
    return in_maps


def kernel(**inputs):
    nc = _get_nc()
    in_maps = prepare_in_maps(inputs)
    res = run_bass_kernel_spmd(nc, in_maps, list(range(NC))).results
    out = np.zeros((BS, D), np.float32)
    for c in range(NC):
        out[TB * c:TB * (c + 1)] = res[c]["out_sh"][0:TB]
        out[S + TB * c:S + TB * (c + 1)] = res[c]["out_sh"][TB:T]
    return out.reshape(B, S, D)


# revision 29
# speedup vs baseline: 1.0851x; 1.0545x over previous
"""Trainium2 Bass kernel for nn_DAWNBlock (8-core SPMD), v5.

Decomposition (validated in numpy: proto_check.py, quant_check.py):
  - Token-sharded with batch-interleaved ownership: core c owns global tokens
    [256c, 256c+256) of batch 0 AND [256c, 256c+256) of batch 1 (T=512).
    LN1, features, restores (Q/K/V), LN2 + knowledge run token-local.
  - Head-sharded attention: core c owns d-cols [128c, 128c+128) (heads
    {2c, 2c+1}). One A2A reshards Q^T+K^T (1MB fp8), one reshards V (0.5MB).
    Attention output returns via TWO 0.25MB A2As (one per batch) so batch 0's
    resharding and its W_O hide under batch 1's attention.
  - fp8e4m3 + DoubleRow (K=256 pairs, 2x PE rate) for features, restores,
    PV, W_O, knowledge. Scores fp8 K=64 packed as concurrent row-group pairs.
    All-fp8 quantization study: 4e-3 rel err vs the 2e-2 gate.
  - LN gains folded into the f-pools host-side (bias asserted zero); LN is
    stats + one ACT apply. Routing weights: feature PSUM banks combined via
    DVE scalar_tensor_tensor; restores use A[n] = hT * wbc[n] (PE-broadcast
    rows), h transposed once.
  - Causal softmax without max-subtraction; denominator via ones-column in V;
    o_ps evacuated to SBUF immediately to release PSUM banks.
"""
import sys

sys.path.insert(0, "/opt/trn_rl_repo")

import os
import numpy as np
import ml_dtypes
import concourse.bass as bass
import concourse.mybir as mybir
import concourse.tile as tile
from concourse import bacc
from concourse.bass_utils import run_bass_kernel_spmd
from concourse.masks import make_identity

B, S, D, H, R, N, KR = 2, 2048, 1024, 16, 256, 8, 128
DH = D // H           # 64
BS = B * S            # 4096
NC = 8
T = BS // NC          # 512 tokens per core (256 per batch)
TB = T // B           # 256 tokens per (core, batch)
P = 128
TT = T // P           # 4 token tiles per core
DC = D // P           # 8 d chunks
DCP = DC // 2         # 4 d chunk-pairs (DoubleRow)
RC = R // P           # 2 r chunks
EPS = 1e-5

STAGES = int(os.environ.get("BASS_STAGES", "5"))
F32 = mybir.dt.float32
BF = mybir.dt.bfloat16
F8 = mybir.dt.float8e4
DR = mybir.MatmulPerfMode.DoubleRow
AF = mybir.ActivationFunctionType
OP = mybir.AluOpType


def _layernorm(nc, cpool, x_sb, nx_sb, eps_tile, tag):
    """nx = (x - mean(x)) * rsqrt(var + eps) for one [128, D] tile.
    (LN gain folded into downstream pools host-side; bias asserted zero.)"""
    s = cpool.tile([P, 1], F32, tag="ln_s", name=f"{tag}_s")
    nm = cpool.tile([P, 1], F32, tag="ln_nm", name=f"{tag}_nm")
    sq = cpool.tile([P, D], F32, tag="ln_sq", name=f"{tag}_sq")
    ssq = cpool.tile([P, 1], F32, tag="ln_ssq", name=f"{tag}_ssq")
    rs = cpool.tile([P, 1], F32, tag="ln_rs", name=f"{tag}_rs")
    nmrs = cpool.tile([P, 1], F32, tag="ln_nmrs", name=f"{tag}_nmrs")
    nc.vector.reduce_sum(s[:], x_sb[:], axis=mybir.AxisListType.X)
    nc.vector.tensor_scalar_mul(nm[:], s[:], -1.0 / D)
    nc.scalar.activation(sq[:], x_sb[:], AF.Square, bias=nm[:], accum_out=ssq[:])
    nc.scalar.activation(rs[:], ssq[:], AF.Abs_reciprocal_sqrt,
                         bias=eps_tile[:], scale=1.0 / D)
    nc.vector.tensor_mul(nmrs[:], nm[:], rs[:])
    nc.scalar.activation(nx_sb[:], x_sb[:], AF.Identity, bias=nmrs[:], scale=rs[:])


def _build():
    nc = bacc.Bacc("TRN2", target_bir_lowering=False, debug=False, num_devices=NC)

    def di(name, shape, dt=F8):
        return nc.dram_tensor(name, shape, dt, kind="ExternalInput").ap()

    x_in = di("x_sh", [T, D], F32)
    wcol_in = {k: di(k, [T, N], F32) for k in ["wfq", "wfk", "wfv", "wkf"]}
    wbc_in = {k: di(k, [N, P, T], BF) for k in ["wbcq", "wbck", "wbcv", "wbckr"]}
    fqk_in = di("fqk_p", [N, D, R])
    fv_in = di("fv_p", [N, D, R])
    rqk_in = di("rqk_p", [N, R, D])
    rv_in = di("rv_p", [N, R, D])
    fkn_in = di("fkn_p", [N, D, KR])
    rkn_in = di("rkn_p", [N, KR, D])
    wo_in = di("wo_p", [D, D])       # = W_O.T
    masku_in = di("masku", [P, P])
    out_ap = nc.dram_tensor("out_sh", [T, D], F32, kind="ExternalOutput").ap()

    with tile.TileContext(nc) as tc:
        from contextlib import ExitStack
        with ExitStack() as ctx:
            const = ctx.enter_context(tc.tile_pool(name="const", bufs=1))
            cpool = ctx.enter_context(tc.tile_pool(name="scratch", bufs=2))
            dram = ctx.enter_context(tc.tile_pool(name="dram", bufs=1, space="DRAM"))

            # ---------- pools (stack order: long-lived first) ----------
            xpool = ctx.enter_context(tc.tile_pool(name="xpool", bufs=1))
            x_t = [xpool.tile([P, D], F32, tag=f"x{tt}", name=f"x{tt}") for tt in range(TT)]

            cprep = ctx.enter_context(tc.tile_pool(name="cprep", bufs=1))
            wo_sb = cprep.tile([P, DCP, 2, D], F8, tag="wo_sb", name="wo_sb")
            fkn_sb = cprep.tile([P, DCP, 2, N, KR], F8, tag="fkn_sb", name="fkn_sb")
            rkn_sb = cprep.tile([P, N, D], F8, tag="rkn_sb", name="rkn_sb")

            hpool = ctx.enter_context(tc.tile_pool(name="hpool", bufs=1))
            h_q = [hpool.tile([P, R], F32, tag=f"hq{tt}", name=f"hq{tt}") for tt in range(TT)]
            h_k = [hpool.tile([P, R], F32, tag=f"hk{tt}", name=f"hk{tt}") for tt in range(TT)]
            h_v = [hpool.tile([P, R], F32, tag=f"hv{tt}", name=f"hv{tt}") for tt in range(TT)]
            hbf_q = [hpool.tile([P, R], BF, tag=f"hbq{tt}", name=f"hbq{tt}") for tt in range(TT)]
            hbf_k = [hpool.tile([P, R], BF, tag=f"hbk{tt}", name=f"hbk{tt}") for tt in range(TT)]
            hbf_v = [hpool.tile([P, R], BF, tag=f"hbv{tt}", name=f"hbv{tt}") for tt in range(TT)]

            wpool = ctx.enter_context(tc.tile_pool(name="wpool", bufs=1))
            wbc = {}
            for k in ["wbcq", "wbck", "wbcv"]:
                wbc[k] = [wpool.tile([P, T], BF, tag=f"{k}{n}", name=f"{k}{n}")
                          for n in range(N)]
            wbc_kr = [wpool.tile([P, T], BF, tag=f"wbckr{n}", name=f"wbckr{n}")
                      for n in range(N)]

            from contextlib import ExitStack as _ES
            rp_ctx = _ES()
            rp = rp_ctx.enter_context(tc.tile_pool(name="rp", bufs=1))
            rqk_sb = rp.tile([P, N, RC, D], F8, tag="rqk_sb", name="rqk_sb")
            rv_sb = rp.tile([P, N, RC, D], F8, tag="rv_sb", name="rv_sb")

            fw_ctx = _ES()
            fwpool = fw_ctx.enter_context(tc.tile_pool(name="fwpool", bufs=1))
            fqk_sb = [fwpool.tile([P, 2, N, R], F8, tag=f"fqk{dcp}", name=f"fqk{dcp}")
                      for dcp in range(DCP)]
            fv_sb = [fwpool.tile([P, 2, N, R], F8, tag=f"fv{dcp}", name=f"fv{dcp}")
                     for dcp in range(DCP)]

            # ---------- big prefetches in priority order ----------
            for tt in range(TT):
                nc.sync.dma_start(x_t[tt][:], x_in[P * tt:P * (tt + 1), :])
            for dcp in range(DCP):
                for i in range(2):
                    dc = 2 * dcp + i
                    nc.sync.dma_start(
                        fqk_sb[dcp][:, i],
                        fqk_in[:, P * dc:P * (dc + 1), :].rearrange("n p r -> p n r"))
                    nc.sync.dma_start(
                        fv_sb[dcp][:, i],
                        fv_in[:, P * dc:P * (dc + 1), :].rearrange("n p r -> p n r"))
            for n in range(N):
                for k in ["wbcq", "wbck", "wbcv"]:
                    nc.sync.dma_start(wbc[k][n][:], wbc_in[k][n])
                nc.sync.dma_start(wbc_kr[n][:], wbc_in["wbckr"][n])
            for n in range(N):
                nc.sync.dma_start(
                    rqk_sb[:, n], rqk_in[n].rearrange("(rc p) d -> p rc d", p=P))
                nc.sync.dma_start(
                    rv_sb[:, n], rv_in[n].rearrange("(rc p) d -> p rc d", p=P))
            # tiny warm-up collective: pays the first-collective ncfw latency
            dumm_b = dram.tile([NC * 8, 8], F32, tag="dumm_b", name="dumm_b")
            dumm_o = dram.tile([NC * 8, 8], F32, tag="dumm_o", name="dumm_o")
            nc.gpsimd.collective_compute(
                "AllToAll", OP.bypass, replica_groups=[list(range(NC))],
                ins=[dumm_b.opt()], outs=[dumm_o.opt()])

            ident_f = const.tile([P, P], F32, tag="ident_f", name="ident_f")
            make_identity(nc, ident_f)
            identb = const.tile([P, P], BF, tag="identb", name="identb")
            nc.vector.tensor_copy(identb[:], ident_f[:])
            ones_bf = const.tile([1, P], BF, tag="ones_bf", name="ones_bf")
            nc.vector.memset(ones_bf[:], 1.0)
            masku = const.tile([P, P], F8, tag="masku", name="masku")
            nc.sync.dma_start(masku[:], masku_in[:])
            eps_t = const.tile([P, 1], F32, tag="eps", name="eps")
            nc.vector.memset(eps_t[:], EPS)

            wsb = {}
            for k in wcol_in:
                wt = const.tile([P, TT, N], F32, tag=f"w_{k}", name=f"w_{k}")
                nc.sync.dma_start(wt[:], wcol_in[k].rearrange("(tt p) n -> p tt n", p=P))
                wsb[k] = wt

            # ---------- DRAM bounce buffers for collectives ----------
            qk_b = dram.tile([NC * 2 * P, T], F8, tag="qk_b", name="qk_b")
            qk_o = dram.tile([NC * 2 * P, T], F8, tag="qk_o", name="qk_o")
            v_b = dram.tile([BS, P], F8, tag="v_b", name="v_b")
            v_o = dram.tile([BS, P], F8, tag="v_o", name="v_o")
            ab_bs = [dram.tile([NC * P, TB], F8, tag=f"ab_b{b}", name=f"ab_b{b}")
                     for b in range(B)]
            ab_os = [dram.tile([NC * P, TB], F8, tag=f"ab_o{b}", name=f"ab_o{b}")
                     for b in range(B)]

            # ================= stage A: LN1 + transpose + features ========
            with tc.tile_pool(name="stagea", bufs=1) as stagea:
                nxT = {}
                with tc.tile_pool(name="ps_tr", bufs=2, space="PSUM") as ps_tr:
                    for tt in range(TT):
                        nx_t = stagea.tile([P, D], BF, tag="nxa", name=f"nxa{tt}", bufs=2)
                        _layernorm(nc, cpool, x_t[tt], nx_t, eps_t, f"ln1_{tt}")
                        for dcp in range(DCP):
                            pst = ps_tr.tile([P, 2, P], BF, tag="tp", name="tp")
                            for i in range(2):
                                dc = 2 * dcp + i
                                nc.tensor.transpose(pst[:, i], nx_t[:, P * dc:P * (dc + 1)],
                                                    identb[:])
                            t8 = stagea.tile([P, 2, P], F8, tag=f"nxT{tt}_{dcp}",
                                             name=f"nxT{tt}_{dcp}")
                            nc.vector.tensor_copy(t8[:], pst[:])
                            nxT[(tt, dcp)] = t8

                with tc.tile_pool(name="ps_feat", bufs=8, space="PSUM") as ps_feat:
                    for tt in range(TT):
                        psf = [ps_feat.tile([P, 512], F32, tag="feat", name=f"feat{tt}_{g}")
                               for g in range(8)]
                        for dcp in range(DCP):
                            lhs = nxT[(tt, dcp)][:]
                            for g in range(8):
                                src = (fqk_sb if g < 4 else fv_sb)[dcp]
                                gg = g % 4
                                nc.tensor.matmul(
                                    psf[g][:], lhs, src[:, :, 2 * gg:2 * (gg + 1), :],
                                    start=(dcp == 0), stop=(dcp == DCP - 1), perf_mode=DR)
                        for m in range(N):
                            bank, half = m // 2, m % 2
                            pq = psf[bank][:, R * half:R * (half + 1)]
                            pv = psf[4 + bank][:, R * half:R * (half + 1)]
                            for htiles, hbfs, wk, ps_slice in (
                                    (h_q, hbf_q, "wfq", pq), (h_k, hbf_k, "wfk", pq),
                                    (h_v, hbf_v, "wfv", pv)):
                                wcol = wsb[wk][:, tt, m:m + 1]
                                if m == 0:
                                    nc.vector.tensor_scalar_mul(htiles[tt][:], ps_slice, wcol)
                                elif m == N - 1:
                                    nc.vector.scalar_tensor_tensor(
                                        hbfs[tt][:], ps_slice, wcol, htiles[tt][:],
                                        op0=OP.mult, op1=OP.add)
                                else:
                                    nc.vector.scalar_tensor_tensor(
                                        htiles[tt][:], ps_slice, wcol, htiles[tt][:],
                                        op0=OP.mult, op1=OP.add)

            fw_ctx.close()

            if STAGES == 1:
                for tt in range(TT):
                    hq_ev = cpool.tile([P, R], F32, tag="hq_ev", name=f"hq_ev{tt}")
                    nc.vector.tensor_copy(hq_ev[:], h_q[tt][:])
                    nc.sync.dma_start(out_ap[P * tt:P * (tt + 1), 0:R], hq_ev[:])

            # ================= stage B: restores + A2A =====================
            if STAGES >= 2:
                with tc.tile_pool(name="bpool", bufs=1) as bpool, \
                     tc.tile_pool(name="ap_pool", bufs=2) as ap_pool, \
                     tc.tile_pool(name="ps_bt", bufs=2, space="PSUM") as ps_bt, \
                     tc.tile_pool(name="ps_r", bufs=6, space="PSUM") as ps_r, \
                     tc.tile_pool(name="ev_b", bufs=3) as ev_b:
                    hT = {}
                    for key, hbfs in (("q", hbf_q), ("k", hbf_k), ("v", hbf_v)):
                        ht = bpool.tile([P, RC, T], BF, tag=f"hT{key}", name=f"hT{key}")
                        for tt in range(TT):
                            pst = ps_bt.tile([P, 2, P], BF, tag="tpb", name="tpb")
                            for rc in range(RC):
                                nc.tensor.transpose(pst[:, rc],
                                                    hbfs[tt][:, P * rc:P * (rc + 1)],
                                                    identb[:])
                            nc.vector.tensor_copy(ht[:, :, P * tt:P * (tt + 1)], pst[:])
                        hT[key] = ht

                    def build_A(hkey, wkey):  # wkey in wbcq/wbck/wbcv
                        A = [ap_pool.tile([P, RC, T], F8, tag=f"A{n}", name=f"A_{wkey}{n}")
                             for n in range(N)]
                        for n in range(N):
                            for rc in range(RC):
                                nc.vector.tensor_mul(
                                    A[n][:, rc, :], hT[hkey][:, rc, :], wbc[wkey][n][:])
                        return A

                    def qk_restore(A, row_off):
                        for dm in range(DC):
                            ps = ps_r.tile([P, T], F32, tag="r_ps", name="r_ps")
                            for n in range(N):
                                nc.tensor.matmul(
                                    ps[:], rqk_sb[:, n, :, P * dm:P * (dm + 1)], A[n][:],
                                    start=(n == 0), stop=(n == N - 1), perf_mode=DR)
                            ev = ev_b.tile([P, T], F8, tag="ev_qk", name="ev_qk")
                            nc.scalar.activation(ev[:], ps[:], AF.Copy)
                            nc.sync.dma_start(
                                qk_b[2 * P * dm + row_off: 2 * P * dm + row_off + P, :],
                                ev[:])

                    Aq = build_A("q", "wbcq")
                    Ak = build_A("k", "wbck")
                    Av = build_A("v", "wbcv")
                    qk_restore(Aq, 0)
                    qk_restore(Ak, P)
                    cc_qk = nc.gpsimd.collective_compute(
                        "AllToAll", OP.bypass, replica_groups=[list(range(NC))],
                        ins=[qk_b.opt()], outs=[qk_o.opt()])
                    first_v_mm = [None]
                    for tt in range(TT):
                        for jf in range(2):
                            ps = ps_r.tile([P, 512], F32, tag="r_ps", name="v_ps")
                            for n in range(N):
                                mm = nc.tensor.matmul(
                                    ps[:], Av[n][:, :, P * tt:P * (tt + 1)],
                                    rv_sb[:, n, :, 512 * jf:512 * (jf + 1)],
                                    start=(n == 0), stop=(n == N - 1), perf_mode=DR)
                                if first_v_mm[0] is None:
                                    first_v_mm[0] = mm
                            ev = ev_b.tile([P, 512], F8, tag="ev_v", name="ev_v")
                            nc.vector.tensor_copy(ev[:], ps[:])
                            for db in range(4):
                                d = 4 * jf + db
                                nc.sync.dma_start(
                                    v_b[T * d + P * tt: T * d + P * (tt + 1), :],
                                    ev[:, P * db:P * (db + 1)])
                    nc.gpsimd.collective_compute(
                        "AllToAll", OP.bypass, replica_groups=[list(range(NC))],
                        ins=[v_b.opt()], outs=[v_o.opt()])
                    # hold V-restore matmuls until the qk collective trigger has
                    # fired, so they execute inside its completion window
                    from concourse.tile import add_dep_helper as _adh
                    try:
                        _adh(cc_qk.ins if hasattr(cc_qk, "ins") else cc_qk,
                             first_v_mm[0].ins if hasattr(first_v_mm[0], "ins")
                             else first_v_mm[0],
                             sync=True, reason="V restore covers qk A2A window")
                    except Exception:
                        pass
            rp_ctx.close()

            if STAGES == 2:
                for tt in range(TT):
                    qo_ev = cpool.tile([P, T], F8, tag="qo_ev", name=f"qo_ev{tt}")
                    nc.sync.dma_start(qo_ev[:], qk_o[2 * P * tt:2 * P * tt + P, :])
                    qo_f = cpool.tile([P, T], F32, tag="qo_f", name=f"qo_f{tt}")
                    nc.vector.tensor_copy(qo_f[:], qo_ev[:])
                    nc.sync.dma_start(out_ap[P * tt:P * (tt + 1), 0:T], qo_f[:])

            # ========== attention (head-sharded, packed) + W_O + stage C ===
            if STAGES >= 3:
                cpers = ctx.enter_context(tc.tile_pool(name="cpers", bufs=1))
                cscr = ctx.enter_context(tc.tile_pool(name="cscr", bufs=2))
                ps_acc = ctx.enter_context(
                    tc.tile_pool(name="ps_acc", bufs=2, space="PSUM"))
                aTs = [cpers.tile([P, DCP, 2, TB], F8, tag=f"aT{b}", name=f"aT{b}")
                       for b in range(B)]
                x2 = [cpers.tile([P, D], F32, tag=f"x2_{tt}", name=f"x2_{tt}")
                      for tt in range(TT)]

                def wo_block(tt):
                    """W_O for token tile tt (tokens of batch tt//2)."""
                    hb, off = tt // 2, P * (tt % 2)
                    for jf in range(2):
                        ps = ps_acc.tile([P, 512], F32, tag="acc", name=f"wo_ps{tt}")
                        for dcp in range(DCP):
                            nc.tensor.matmul(
                                ps[:], aTs[hb][:, dcp, :, off:off + P],
                                wo_sb[:, dcp, :, 512 * jf:512 * (jf + 1)],
                                start=(dcp == 0), stop=(dcp == DCP - 1), perf_mode=DR)
                        nc.vector.tensor_add(
                            x2[tt][:, 512 * jf:512 * (jf + 1)],
                            x_t[tt][:, 512 * jf:512 * (jf + 1)], ps[:])

                with tc.tile_pool(name="qkv_bh", bufs=1) as qkv_bh, \
                     tc.tile_pool(name="pt_pool", bufs=20) as pt_pool, \
                     tc.tile_pool(name="osb_pool", bufs=4) as osb_pool, \
                     tc.tile_pool(name="ps_st", bufs=4, space="PSUM") as ps_st, \
                     tc.tile_pool(name="ps_o", bufs=2, space="PSUM") as ps_o:
                    # all q/k loads first (vp loads wait on the V collective and
                    # would otherwise block batch-1 q/k behind them in the queue);
                    # fine-grained tiles so early score matmuls start per-chunk
                    qtg, ktc, vpss = {}, {}, []
                    for b in range(B):
                        for qg in range(4):
                            t = qkv_bh.tile([P, 512], F8, tag=f"qtg{b}_{qg}",
                                            name=f"qtg{b}_{qg}")
                            for half in range(2):
                                c = 2 * qg + half
                                nc.sync.dma_start(
                                    t[:, TB * half:TB * (half + 1)],
                                    qk_o[2 * P * c:2 * P * c + P, TB * b:TB * (b + 1)])
                            qtg[(b, qg)] = t
                        for c in range(NC):
                            kt_t = qkv_bh.tile([P, TB], F8, tag=f"ktc{b}_{c}",
                                               name=f"ktc{b}_{c}")
                            nc.sync.dma_start(
                                kt_t[:], qk_o[2 * P * c + P:2 * P * (c + 1),
                                              TB * b:TB * (b + 1)])
                            ktc[(b, c)] = kt_t
                    for b in range(B):
                        vps = []
                        for h2 in range(2):
                            vp = qkv_bh.tile([P, S // P // 2, 2, 80], F8,
                                             tag=f"vp{h2}_{b}", name=f"vp{h2}_{b}")
                            src = v_o.rearrange("(c hb i p) f -> hb p c i f",
                                                hb=2, i=2, p=P)[b]
                            for i in range(2):
                                nc.sync.dma_start(
                                    vp[:, :, i, 0:DH],
                                    src[:, :, i, DH * h2:DH * (h2 + 1)])
                            nc.vector.memset(vp[:, :, :, DH:DH + 1], 1.0)
                            vps.append(vp)
                        vpss.append(vps)
                    # stage-C weights: needed only after attention; emitted after
                    # the attention loads so they don't block those DMA queues
                    for dcp in range(DCP):
                        for i in range(2):
                            dc = 2 * dcp + i
                            nc.sync.dma_start(wo_sb[:, dcp, i],
                                              wo_in[P * dc:P * (dc + 1), :])
                            nc.sync.dma_start(
                                fkn_sb[:, dcp, i],
                                fkn_in[:, P * dc:P * (dc + 1), :].rearrange("n p f -> p n f"))
                    for n in range(N):
                        nc.sync.dma_start(rkn_sb[:, n], rkn_in[n])

                    for b in range(B):
                        vps = vpss[b]
                        for qg in range(4):
                            o_ps = [ps_o.tile([DH + 1, 512], F32, tag="o_ps",
                                              name=f"o{b}_{qg}_{h2}") for h2 in range(2)]
                            nkt = 4 * qg + 4
                            for u in range(nkt // 2):
                                pt2 = [pt_pool.tile([P, 2, 512], F8, tag="pt",
                                                    name=f"pt{b}_{qg}_{u}_{h2}")
                                       for h2 in range(2)]
                                j0 = 2 * u - 4 * qg   # j of the pair's first chunk
                                vs = max(0, P * j0)   # valid query start for the pair
                                for i in range(2):
                                    kt = 2 * u + i
                                    j = kt - 4 * qg
                                    qs = max(0, P * j)  # this chunk's valid query start
                                    for h2 in range(2):
                                        st = ps_st.tile([P, 512], F32, tag="st", name="st")
                                        koff = P * (kt % 2)
                                        nc.tensor.matmul(
                                            st[:, qs:],
                                            ktc[(b, kt // 2)][DH * h2:DH * (h2 + 1),
                                                              koff:koff + P],
                                            qtg[(b, qg)][DH * h2:DH * (h2 + 1), qs:])
                                        pt = pt2[h2][:, i, :]
                                        if j < 0:
                                            nc.scalar.activation(pt, st[:], AF.Exp,
                                                                 scale=0.125)
                                        else:
                                            if qs > vs:
                                                # zero only the slice PV will read
                                                nc.vector.memset(pt[:, vs:qs], 0.0)
                                            nc.scalar.activation(
                                                pt[:, qs:], st[:, qs:],
                                                AF.Exp, scale=0.125)
                                            nc.vector.tensor_mul(
                                                pt[:, P * j:P * (j + 1)],
                                                pt[:, P * j:P * (j + 1)], masku[:])
                                for h2 in range(2):
                                    nc.tensor.matmul(
                                        o_ps[h2][:, vs:], vps[h2][:, u, :, 0:DH + 1],
                                        pt2[h2][:, :, vs:],
                                        start=(u == 0), stop=(u == nkt // 2 - 1),
                                        perf_mode=DR, skip_group_check=True)
                            for h2 in range(2):
                                # evacuate PSUM immediately; normalize from SBUF
                                den = cpool.tile([1, 512], F32, tag="den", name="den")
                                nc.vector.tensor_copy(den[:], o_ps[h2][DH:DH + 1, :])
                                o_sb = osb_pool.tile([DH, 512], BF, tag="o_sb",
                                                     name=f"osb{b}_{qg}_{h2}")
                                nc.vector.tensor_copy(o_sb[:], o_ps[h2][0:DH, :])
                                rec_f = cpool.tile([1, 512], F32, tag="rec_f", name="rec_f")
                                nc.vector.reciprocal_approx_fast(rec_f[:], den[:])
                                rec = cpool.tile([1, 512], BF, tag="rec", name="rec")
                                nc.vector.tensor_copy(rec[:], rec_f[:])
                                bc = ps_st.tile([DH, 512], F32, tag="st", name="bc")
                                nc.tensor.matmul(bc[:], ones_bf[:, 0:DH], rec[:])
                                bc_sb = cpool.tile([DH, 512], BF, tag="bc_sb", name="bc_sb")
                                nc.vector.tensor_copy(bc_sb[:], bc[:])
                                nrm = cpool.tile([DH, 512], F8, tag="nrm", name="nrm")
                                nc.vector.tensor_mul(nrm[:], o_sb[:], bc_sb[:])
                                for half in range(2):
                                    cblk = 2 * qg + half
                                    nc.sync.dma_start(
                                        ab_bs[b][P * cblk + DH * h2:
                                                 P * cblk + DH * (h2 + 1), :],
                                        nrm[:, TB * half:TB * (half + 1)])
                        nc.gpsimd.collective_compute(
                            "AllToAll", OP.bypass, replica_groups=[list(range(NC))],
                            ins=[ab_bs[b].opt()], outs=[ab_os[b].opt()])
                        for dcp in range(DCP):
                            for i in range(2):
                                dc = 2 * dcp + i
                                nc.sync.dma_start(aTs[b][:, dcp, i],
                                                  ab_os[b][P * dc:P * (dc + 1), :])
                        if b == 1:
                            # W_O for batch 0 tokens: fills the PE while the
                            # batch-1 A2A completes
                            wo_block(0)
                            wo_block(1)

                if STAGES == 3:
                    for tt in range(TT):
                        at_f = cpool.tile([P, D], F32, tag="at_f", name=f"at_f{tt}")
                        nc.vector.tensor_copy(at_f[:], x2[tt % 2][:])
                        nc.sync.dma_start(out_ap[P * tt:P * (tt + 1), :], at_f[:])

                # ---------------- stage C ----------------
                if STAGES >= 4:
                    with tc.tile_pool(name="ps_ct", bufs=2, space="PSUM") as ps_ct, \
                         tc.tile_pool(name="ps_kf", bufs=2, space="PSUM") as ps_kf, \
                         tc.tile_pool(name="ps_kr", bufs=2, space="PSUM") as ps_kr:
                        wo_block(2)
                        wo_block(3)
                        nx2T = {}
                        hknT = cpers.tile([P, T], BF, tag="hknT", name="hknT")
                        for tt in range(TT):
                            if STAGES >= 5:
                                nx2 = cscr.tile([P, D], BF, tag="nx2", name=f"nx2_{tt}")
                                _layernorm(nc, cpool, x2[tt], nx2, eps_t, f"ln2_{tt}")
                                for dcp in range(DCP):
                                    pst = ps_ct.tile([P, 2, P], BF, tag="tpc", name="tpc")
                                    for i in range(2):
                                        dc = 2 * dcp + i
                                        nc.tensor.transpose(
                                            pst[:, i], nx2[:, P * dc:P * (dc + 1)], identb[:])
                                    t8 = cscr.tile([P, 2, P], F8, tag=f"nx2T{tt}_{dcp}",
                                                   name=f"nx2T{tt}_{dcp}", bufs=1)
                                    nc.vector.tensor_copy(t8[:], pst[:])
                                    nx2T[(tt, dcp)] = t8

                        if STAGES == 4:
                            for tt in range(TT):
                                ao_ev = cpool.tile([P, D], F32, tag="ao_ev", name=f"ao_ev{tt}")
                                nc.vector.tensor_copy(ao_ev[:], x2[tt][:])
                                nc.sync.dma_start(out_ap[P * tt:P * (tt + 1), :], ao_ev[:])

                        if STAGES >= 5:
                            for tt in range(TT):
                                psk = [ps_kf.tile([P, 512], F32, tag="kf", name=f"kf{tt}_{g}")
                                       for g in range(2)]
                                for dcp in range(DCP):
                                    lhs = nx2T[(tt, dcp)][:]
                                    for g in range(2):
                                        nc.tensor.matmul(
                                            psk[g][:], lhs,
                                            fkn_sb[:, dcp, :, 4 * g:4 * (g + 1), :],
                                            start=(dcp == 0), stop=(dcp == DCP - 1),
                                            perf_mode=DR)
                                hkn = cscr.tile([P, KR], F32, tag="hkn", name=f"hkn{tt}")
                                for m in range(N):
                                    pslice = psk[m // 4][:, KR * (m % 4):KR * (m % 4 + 1)]
                                    wcol = wsb["wkf"][:, tt, m:m + 1]
                                    if m == 0:
                                        nc.vector.tensor_scalar_mul(hkn[:], pslice, wcol)
                                    else:
                                        nc.vector.scalar_tensor_tensor(
                                            hkn[:], pslice, wcol, hkn[:],
                                            op0=OP.mult, op1=OP.add)
                                hknb = cscr.tile([P, KR], BF, tag="hknb", name=f"hknb{tt}")
                                nc.vector.tensor_copy(hknb[:], hkn[:])
                                pst = ps_ct.tile([P, P], BF, tag="tpc", name="tpc_kn")
                                nc.tensor.transpose(pst[:], hknb[:], identb[:])
                                nc.vector.tensor_copy(hknT[:, P * tt:P * (tt + 1)], pst[:])

                            Akn = cpers.tile([P, N, T], F8, tag="Akn", name="Akn")
                            for n in range(N):
                                nc.vector.tensor_mul(Akn[:, n, :], hknT[:], wbc_kr[n][:])
                            for tt in range(TT):
                                for jf in range(2):
                                    ps = ps_kr.tile([P, 512], F32, tag="kr_ps", name="kn_ps")
                                    for u in range(N // 2):
                                        nc.tensor.matmul(
                                            ps[:],
                                            Akn[:, 2 * u:2 * (u + 1), P * tt:P * (tt + 1)],
                                            rkn_sb[:, 2 * u:2 * (u + 1),
                                                   512 * jf:512 * (jf + 1)],
                                            start=(u == 0), stop=(u == N // 2 - 1),
                                            perf_mode=DR)
                                    out_sb = cscr.tile([P, 512], F32, tag="out_sb",
                                                       name="out_sb")
                                    nc.vector.tensor_add(
                                        out_sb[:], x2[tt][:, 512 * jf:512 * (jf + 1)], ps[:])
                                    nc.sync.dma_start(
                                        out_ap[P * tt:P * (tt + 1),
                                               512 * jf:512 * (jf + 1)],
                                        out_sb[:])

    nc.compile()
    return nc


_NC = None


def _get_nc():
    global _NC
    if _NC is None:
        _NC = _build()
    return _NC


def _tok_idx(c):
    """Global token indices owned by core c (batch-interleaved)."""
    return np.r_[TB * c:TB * (c + 1), S * 1 * B // 2 + TB * c:2048 + TB * (c + 1)]


def prepare_in_maps(inputs):
    bf = ml_dtypes.bfloat16
    f8 = ml_dtypes.float8_e4m3
    inp = {k: np.ascontiguousarray(np.asarray(v, dtype=np.float32)) for k, v in inputs.items()}
    x_flat = inp["x"].reshape(BS, D)
    wcols = {
        "wfq": inp["fqk_w_Q"].reshape(BS, N), "wfk": inp["fqk_w_K"].reshape(BS, N),
        "wfv": inp["fv_w"].reshape(BS, N), "wkf": inp["feature_know_w"].reshape(BS, N),
    }
    wrows = {
        "wbcq": inp["rqk_w_Q"].reshape(BS, N), "wbck": inp["rqk_w_K"].reshape(BS, N),
        "wbcv": inp["rv_w"].reshape(BS, N), "wbckr": inp["restore_know_w"].reshape(BS, N),
    }
    g1 = inp["ln1_g"][None, :, None]
    g2 = inp["ln2_g"][None, :, None]
    assert np.abs(inp["ln1_b"]).max() == 0 and np.abs(inp["ln2_b"]).max() == 0, \
        "nonzero LN bias not supported by this build"
    pools = {
        "fqk_p": (inp["f_qk"] * g1).astype(f8), "fv_p": (inp["f_v"] * g1).astype(f8),
        "rqk_p": inp["r_qk"].astype(f8), "rv_p": inp["r_v"].astype(f8),
        "fkn_p": (inp["f_know"] * g2).astype(f8), "rkn_p": inp["r_know"].astype(f8),
    }
    wo_p = np.ascontiguousarray(inp["W_O"].T).astype(f8)
    masku = np.ascontiguousarray(np.tril(np.ones((P, P), np.float32)).T).astype(f8)

    in_maps = []
    for c in range(NC):
        idx = np.r_[TB * c:TB * (c + 1), S + TB * c:S + TB * (c + 1)]
        m = {
            "x_sh": np.ascontiguousarray(x_flat[idx]),
            "wo_p": wo_p, "masku": masku,
        }
        m.update(pools)
        for k, v in wcols.items():
            m[k] = np.ascontiguousarray(v[idx])
        for k, v in wrows.items():
            rowsT = v[idx].T.astype(bf)                       # [N, T]
            m[k] = np.ascontiguousarray(
                np.broadcast_to(rowsT[:, None, :], (N, P, T)))  # [N, P, T]
        in_maps.append(m)
    return in_maps


def kernel(**inputs):
    nc = _get_nc()
    in_maps = prepare_in_maps(inputs)
    res = run_bass_kernel_spmd(nc, in_maps, list(range(NC))).results
    out = np.zeros((BS, D), np.float32)
    for c in range(NC):
        out[TB * c:TB * (c + 1)] = res[c]["out_sh"][0:TB]
        out[S + TB * c:S + TB * (c + 1)] = res[c]["out_sh"][TB:T]
    return out.reshape(B, S, D)
